# revision 11
# baseline (speedup 1.0000x reference)
"""Trainium2 Bass kernel for prior-fused cross-attention.

Math (per batch b, head h):
  q  = query @ (Wq * Dh^-0.5)            # scale folded into Wq on host
  k,v = split(memory @ Wkv)
  s  = q_h @ k_h^T                       # == attn_score output (mask all-ones)
  L  = ln(prior + eps)
  P  = softmax(s + L)                    # == softmax(log_softmax(s) + L)
  y  = P_h @ v_h ;  out = y @ Wo

Layout strategy (per core = one batch element):
  - matmuls in float32r (1 cyc/row at N>=512); every matmul operand tile is
    declared float32r so its producer (copy/activation/DMA) does the rounding
    the BIR verifier demands.
  - s computed in natural layout [t1-part, t2-free]; L added into PSUM via an
    identity-weight matmul; exp on ACT with accum_out giving the softmax
    denominator Z for free; P normalized with a per-partition tensor_scalar.
  - P transposed on PE (per 128x128 block, fp32 mode) to feed the P@v matmul;
    y kept transposed [c-part, t1-free] so the Wo matmul consumes it as lhsT.
"""

import numpy as np

B, T1, T2, C, H = 8, 2048, 512, 512, 8
Dh = C // H
EPS = 1e-8
PDIM = 128
NT1 = T1 // PDIM      # 16 t1 tiles
NSPAN = T1 // 512     # 4 t1 spans of 512
NCC = C // PDIM       # 4 contraction chunks
NTC = T2 // PDIM      # 4 t2 chunks
JJ = 512 // PDIM      # 4 t1-tiles per span


def _build_nc():
    from contextlib import ExitStack

    import concourse.bacc as bacc
    import concourse.mybir as mybir
    import concourse.tile as tile
    from concourse.masks import make_identity

    f32 = mybir.dt.float32
    f32r = mybir.dt.float32r
    AF = mybir.ActivationFunctionType

    nc = bacc.Bacc()
    query = nc.dram_tensor("query", [T1, C], f32, kind="ExternalInput")
    memory = nc.dram_tensor("memory", [T2, C], f32, kind="ExternalInput")
    prior = nc.dram_tensor("prior", [T1, T2], f32, kind="ExternalInput")
    # weights are consumed directly as fp32r matmul operands
    wq = nc.dram_tensor("wq", [C, C], f32r, kind="ExternalInput")  # pre-scaled
    wkv = nc.dram_tensor("wkv", [C, 2 * C], f32r, kind="ExternalInput")
    wo = nc.dram_tensor("wo", [C, C], f32r, kind="ExternalInput")
    out = nc.dram_tensor("out", [T1, C], f32, kind="ExternalOutput")
    prob = nc.dram_tensor("prob", [H, T1, T2], f32, kind="ExternalOutput")
    score = nc.dram_tensor("score", [H, T1, T2], f32, kind="ExternalOutput")

    with tile.TileContext(nc) as tc, ExitStack() as ctx:
        singles = ctx.enter_context(tc.tile_pool(name="singles", bufs=1))
        ps_s = ctx.enter_context(tc.tile_pool(name="ps_s", bufs=3, space="PSUM"))
        ps_t = ctx.enter_context(tc.tile_pool(name="ps_t", bufs=3, space="PSUM"))
        ps_y = ctx.enter_context(tc.tile_pool(name="ps_y", bufs=2, space="PSUM"))

        ident = singles.tile([PDIM, PDIM], f32)
        make_identity(nc, ident[:])
        identr = singles.tile([PDIM, PDIM], f32r)
        nc.vector.tensor_copy(identr[:], ident[:])
        eps_b = singles.tile([PDIM, 1], f32)
        nc.vector.memset(eps_b[:], EPS)

        # ---------------- phase A: weights, transposes, projections --------
        wo_sb = singles.tile([PDIM, NCC, C], f32r)
        nc.sync.dma_start(wo_sb[:], wo[:].rearrange("(cc p) j -> p cc j", p=PDIM))
        kT = singles.tile([PDIM, NCC, T2], f32r)
        v_sb = singles.tile([PDIM, NTC, C], f32r)
        L_sb = singles.tile([PDIM, NT1, T2], f32r)
        qT = singles.tile([PDIM, NCC, T1], f32r)

        ctxA = ctx.enter_context(ExitStack())
        inA = ctxA.enter_context(tc.tile_pool(name="inA", bufs=3))
        bigA = ctxA.enter_context(tc.tile_pool(name="bigA", bufs=1))
        wq_sb = bigA.tile([PDIM, NCC, C], f32r, tag="wq")
        nc.sync.dma_start(wq_sb[:], wq[:].rearrange("(cc p) j -> p cc j", p=PDIM))
        wkv_sb = bigA.tile([PDIM, NCC, 2 * C], f32r, tag="wkv")
        nc.sync.dma_start(wkv_sb[:], wkv[:].rearrange("(cc p) j -> p cc j", p=PDIM))

        # query^T : [c, t1]
        queryT = bigA.tile([PDIM, NCC, T1], f32r, tag="queryT")
        for jt in range(NT1):
            qt = inA.tile([PDIM, C], f32, tag="ldA")
            nc.sync.dma_start(qt[:], query[jt * PDIM:(jt + 1) * PDIM, :])
            pt = ps_t.tile([PDIM, C], f32, tag="pt")
            for cc in range(NCC):
                nc.tensor.transpose(
                    pt[:, cc * PDIM:(cc + 1) * PDIM],
                    qt[:, cc * PDIM:(cc + 1) * PDIM],
                    ident[:],
                )
            nc.scalar.copy(
                queryT[:, :, jt * PDIM:(jt + 1) * PDIM],
                pt[:].rearrange("p (cc t) -> p cc t", t=PDIM),
            )

        # memory^T : [c, t2]
        memT = bigA.tile([PDIM, NCC, T2], f32r, tag="memT")
        for mt in range(NTC):
            mtl = inA.tile([PDIM, C], f32, tag="ldA")
            nc.sync.dma_start(mtl[:], memory[mt * PDIM:(mt + 1) * PDIM, :])
            pt = ps_t.tile([PDIM, C], f32, tag="pt")
            for cc in range(NCC):
                nc.tensor.transpose(
                    pt[:, cc * PDIM:(cc + 1) * PDIM],
                    mtl[:, cc * PDIM:(cc + 1) * PDIM],
                    ident[:],
                )
            nc.scalar.copy(
                memT[:, :, mt * PDIM:(mt + 1) * PDIM],
                pt[:].rearrange("p (cc t) -> p cc t", t=PDIM),
            )

        # q^T = Wq^T @ query^T : [hp*128, t1]  (head pair hp)
        for hp in range(NCC):
            for sp in range(NSPAN):
                ps = ps_s.tile([PDIM, 512], f32, tag="ps")
                for cc in range(NCC):
                    nc.tensor.matmul(
                        ps[:],
                        wq_sb[:, cc, hp * PDIM:(hp + 1) * PDIM],
                        queryT[:, cc, sp * 512:(sp + 1) * 512],
                        start=(cc == 0),
                        stop=(cc == NCC - 1),
                    )
                nc.scalar.copy(qT[:, hp, sp * 512:(sp + 1) * 512], ps[:])

        # k^T = Wk^T @ memory^T : [hp*128, t2]
        for hp in range(NCC):
            ps = ps_s.tile([PDIM, T2], f32, tag="ps")
            for cc in range(NCC):
                nc.tensor.matmul(
                    ps[:],
                    wkv_sb[:, cc, hp * PDIM:(hp + 1) * PDIM],
                    memT[:, cc, :],
                    start=(cc == 0),
                    stop=(cc == NCC - 1),
                )
            nc.vector.tensor_copy(kT[:, hp, :], ps[:])

        # v (natural) = memory @ Wv : [t2, c]
        for tc_ in range(NTC):
            ps = ps_s.tile([PDIM, C], f32, tag="ps")
            for cc in range(NCC):
                nc.tensor.matmul(
                    ps[:],
                    memT[:, cc, tc_ * PDIM:(tc_ + 1) * PDIM],
                    wkv_sb[:, cc, C:2 * C],
                    start=(cc == 0),
                    stop=(cc == NCC - 1),
                )
            nc.vector.tensor_copy(v_sb[:, tc_, :], ps[:])

        # L = ln(prior + eps) (natural layout)
        for jt in range(NT1):
            ptile = inA.tile([PDIM, T2], f32, tag="ldP")
            nc.sync.dma_start(ptile[:], prior[jt * PDIM:(jt + 1) * PDIM, :])
            nc.scalar.activation(L_sb[:, jt, :], ptile[:], AF.Ln, bias=eps_b[:])

        ctxA.close()

        # ---------------- phase B: attention ------------------------------
        Zbig = singles.tile([PDIM, H * NT1], f32)
        recipZ = singles.tile([PDIM, H * NT1], f32)

        poolB = ctx.enter_context(tc.tile_pool(name="poolB", bufs=1))
        yT = poolB.tile([PDIM, NCC, T1], f32r, tag="yT")

        poolE = ctx.enter_context(tc.tile_pool(name="poolE", bufs=5))
        poolS = ctx.enter_context(tc.tile_pool(name="poolS", bufs=3))
        poolP = ctx.enter_context(tc.tile_pool(name="poolP", bufs=3))
        poolPT = ctx.enter_context(tc.tile_pool(name="poolPT", bufs=2))

        def head_qT(h):
            return qT[(h % 2) * Dh:(h % 2 + 1) * Dh, h // 2, :]

        def head_kT(h):
            return kT[(h % 2) * Dh:(h % 2 + 1) * Dh, h // 2, :]

        for sp in range(NSPAN):
            for h in range(H):
                e_tiles = []
                for jj in range(JJ):
                    jt = sp * JJ + jj
                    cix = h * NT1 + jt
                    pss = ps_s.tile([PDIM, T2], f32, tag="ps")
                    # s = q_h @ k_h^T
                    nc.tensor.matmul(
                        pss[:],
                        head_qT(h)[:, jt * PDIM:(jt + 1) * PDIM],
                        head_kT(h),
                        start=True,
                        stop=True,
                    )
                    s_sb = poolS.tile([PDIM, T2], f32)
                    nc.scalar.copy(s_sb[:], pss[:])
                    nc.sync.dma_start(
                        score[h, jt * PDIM:(jt + 1) * PDIM, :], s_sb[:]
                    )
                    # s += L  (identity-weight matmul accumulate)
                    nc.tensor.matmul(
                        pss[:],
                        identr[:],
                        L_sb[:, jt, :],
                        start=False,
                        stop=True,
                        skip_group_check=True,
                    )
                    e_sb = poolE.tile([PDIM, T2], f32)
                    nc.scalar.activation(
                        e_sb[:], pss[:], AF.Exp,
                        accum_out=Zbig[:, cix:cix + 1],
                    )
                    e_tiles.append(e_sb)
                c0 = h * NT1 + sp * JJ
                nc.vector.reciprocal(
                    recipZ[:, c0:c0 + JJ], Zbig[:, c0:c0 + JJ]
                )
                ptT = poolPT.tile([PDIM, NTC, 512], f32r)
                for jj in range(JJ):
                    jt = sp * JJ + jj
                    cix = h * NT1 + jt
                    p_sb = poolP.tile([PDIM, T2], f32)
                    nc.vector.tensor_scalar_mul(
                        p_sb[:], e_tiles[jj][:], recipZ[:, cix:cix + 1]
                    )
                    nc.sync.dma_start(
                        prob[h, jt * PDIM:(jt + 1) * PDIM, :], p_sb[:]
                    )
                    pst = ps_t.tile([PDIM, T2], f32, tag="pt")
                    for tc_ in range(NTC):
                        nc.tensor.transpose(
                            pst[:, tc_ * PDIM:(tc_ + 1) * PDIM],
                            p_sb[:, tc_ * PDIM:(tc_ + 1) * PDIM],
                            ident[:],
                        )
                    nc.vector.tensor_copy(
                        ptT[:, :, jj * PDIM:(jj + 1) * PDIM],
                        pst[:].rearrange("p (tc t) -> p tc t", t=PDIM),
                    )
                # y^T_h[span] = v_h^T @ P^T
                psy = ps_y.tile([Dh, 512], f32, tag="py")
                for tc_ in range(NTC):
                    nc.tensor.matmul(
                        psy[:],
                        v_sb[:, tc_, h * Dh:(h + 1) * Dh],
                        ptT[:, tc_, :],
                        start=(tc_ == 0),
                        stop=(tc_ == NTC - 1),
                    )
                nc.vector.tensor_copy(
                    yT[(h % 2) * Dh:(h % 2 + 1) * Dh, h // 2,
                       sp * 512:(sp + 1) * 512],
                    psy[:],
                )

        # ---------------- phase C: out = y @ Wo ---------------------------
        poolO = ctx.enter_context(tc.tile_pool(name="poolO", bufs=3))
        for jt in range(NT1):
            ps = ps_s.tile([PDIM, C], f32, tag="ps")
            for cc in range(NCC):
                nc.tensor.matmul(
                    ps[:],
                    yT[:, cc, jt * PDIM:(jt + 1) * PDIM],
                    wo_sb[:, cc, :],
                    start=(cc == 0),
                    stop=(cc == NCC - 1),
                )
            o_sb = poolO.tile([PDIM, C], f32)
            nc.vector.tensor_copy(o_sb[:], ps[:])
            nc.sync.dma_start(out[jt * PDIM:(jt + 1) * PDIM, :], o_sb[:])

    nc.compile()
    return nc


_NC_CACHE = {}


def kernel(query, query_mask, memory, memory_mask, attn_prior, Wq, Wkv, Wo):
    from concourse.bass_utils import run_bass_kernel_spmd

    query = np.asarray(query, dtype=np.float32)
    memory = np.asarray(memory, dtype=np.float32)
    attn_prior = np.asarray(attn_prior, dtype=np.float32)
    scale = np.float32(Dh ** -0.5)
    wq_s = np.ascontiguousarray(np.asarray(Wq, dtype=np.float32) * scale)
    wkv_h = np.ascontiguousarray(np.asarray(Wkv, dtype=np.float32))
    wo_h = np.ascontiguousarray(np.asarray(Wo, dtype=np.float32))

    if "nc" not in _NC_CACHE:
        _NC_CACHE["nc"] = _build_nc()
    nc = _NC_CACHE["nc"]

    in_maps = [
        {
            "query": np.ascontiguousarray(query[b]),
            "memory": np.ascontiguousarray(memory[b]),
            "prior": np.ascontiguousarray(attn_prior[b]),
            "wq": wq_s,
            "wkv": wkv_h,
            "wo": wo_h,
        }
        for b in range(B)
    ]
    res = run_bass_kernel_spmd(nc, in_maps, core_ids=list(range(B)))
    _NC_CACHE["last_result"] = res
    out = np.stack([r["out"] for r in res.results])
    prob = np.stack([r["prob"] for r in res.results])
    score = np.stack([r["score"] for r in res.results])
    return out, prob, score


# revision 52
# speedup vs baseline: 1.2083x; 1.2083x over previous
"""Trainium2 Bass kernel for prior-fused cross-attention.

Math (per batch b, head h):
  q  = query @ (Wq * Dh^-0.5)            # scale folded into Wq on host
  k,v = split(memory @ Wkv)
  s  = q_h @ k_h^T                       # == attn_score output (mask all-ones)
  L  = ln(prior + eps)
  P  = softmax(s + L)                    # == softmax(log_softmax(s) + L)
  y  = P_h @ v_h ;  out = y @ Wo

Layout strategy (per core = one batch element):
  - matmuls in float32r (1 cyc/row at N>=512); every matmul operand tile is
    declared float32r so its producer (copy/activation/DMA) does the rounding
    the BIR verifier demands.
  - fully pipelined over 4 t1-spans of 512: per span, transpose query on PE,
    project qT, ln(prior), then per head: s into one PSUM bank (copied out for
    the attn_score output) and s+L into another (identity-weight matmul
    accumulate) so exp never waits on the score copy/DMA; exp on ACT with
    accum_out giving the softmax denominator Z for free; P normalized with a
    per-partition tensor_scalar (split gpsimd/DVE); P transposed on PE to feed
    the P@v matmul; y kept transposed [c-part, t1-free] so the Wo matmul
    consumes it as lhsT; out-projection folded into each span.
"""

import numpy as np

B, T1, T2, C, H = 8, 2048, 512, 512, 8
Dh = C // H
EPS = 1e-8
PDIM = 128
NT1 = T1 // PDIM      # 16 t1 tiles
NSPAN = T1 // 512     # 4 t1 spans of 512
NCC = C // PDIM       # 4 contraction chunks
NTC = T2 // PDIM      # 4 t2 chunks
JJ = 512 // PDIM      # 4 t1-tiles per span


def _build_nc():
    from contextlib import ExitStack

    import concourse.bacc as bacc
    import concourse.mybir as mybir
    import concourse.tile as tile
    from concourse.masks import make_identity

    f32 = mybir.dt.float32
    f32r = mybir.dt.float32r
    AF = mybir.ActivationFunctionType

    # Prefer the act-function set holding Ln+Exp+Copy together so per-span
    # interleaving of ln/exp/copy needs a single table load.
    import concourse.hw_specs as _hw
    _orig_tabs = _hw.get_activation_tables

    def _tabs(arch, *a, **k):
        # Keep dict order (act_func_set_id = insertion index!) but strip
        # Ln/Exp from every other set so the selector lands on the combined
        # Ln+Exp+Copy set, minimizing runtime table reloads.
        import concourse.mybir as _mb
        tabs = dict(_orig_tabs(arch, *a, **k))
        strip = {_mb.ActivationFunctionType.Ln, _mb.ActivationFunctionType.Exp}
        return {
            name: (fns if name == 'natural_log_exp_and_others'
                   else set(fns) - strip)
            for name, fns in tabs.items()
        }

    bacc.get_activation_tables = _tabs

    nc = bacc.Bacc()
    query = nc.dram_tensor("query", [T1, C], f32, kind="ExternalInput")
    memory = nc.dram_tensor("memory", [T2, C], f32, kind="ExternalInput")
    prior = nc.dram_tensor("prior", [T1, T2], f32, kind="ExternalInput")
    wq = nc.dram_tensor("wq", [C, C], f32r, kind="ExternalInput")  # pre-scaled
    wkv = nc.dram_tensor("wkv", [C, 2 * C], f32r, kind="ExternalInput")
    wo = nc.dram_tensor("wo", [C, C], f32r, kind="ExternalInput")
    out = nc.dram_tensor("out", [T1, C], f32, kind="ExternalOutput")
    prob = nc.dram_tensor("prob", [H, T1, T2], f32, kind="ExternalOutput")
    score = nc.dram_tensor("score", [H, T1, T2], f32, kind="ExternalOutput")

    with tile.TileContext(nc) as tc, ExitStack() as ctx:
        singles = ctx.enter_context(tc.tile_pool(name="singles", bufs=1))
        ps_s = ctx.enter_context(tc.tile_pool(name="ps_s", bufs=2, space="PSUM"))
        ps_e = ctx.enter_context(tc.tile_pool(name="ps_e", bufs=2, space="PSUM"))
        ps_t = ctx.enter_context(tc.tile_pool(name="ps_t", bufs=2, space="PSUM"))
        ps_y = ctx.enter_context(tc.tile_pool(name="ps_y", bufs=2, space="PSUM"))

        ident = singles.tile([PDIM, PDIM], f32)
        make_identity(nc, ident[:])
        identr = singles.tile([PDIM, PDIM], f32r)
        nc.vector.tensor_copy(identr[:], ident[:])
        eps_b = singles.tile([PDIM, 1], f32)
        nc.vector.memset(eps_b[:], EPS)

        kT = singles.tile([PDIM, NCC, T2], f32r)
        v_sb = singles.tile([PDIM, NTC, C], f32r)
        wq_sb = singles.tile([PDIM, NCC, C], f32r)
        wo_sb = singles.tile([PDIM, NCC, C], f32r)

        # main pools first so the kv-header scope can close mid-stream (LIFO)
        poolIn = ctx.enter_context(tc.tile_pool(name="poolIn", bufs=3))
        poolQT = ctx.enter_context(tc.tile_pool(name="poolQT", bufs=2))
        poolL = ctx.enter_context(tc.tile_pool(name="poolL", bufs=2))
        poolB = ctx.enter_context(tc.tile_pool(name="poolB", bufs=1))
        yT = poolB.tile([PDIM, NCC, T1], f32r, tag="yT")
        poolE = ctx.enter_context(tc.tile_pool(name="poolE", bufs=8))
        poolS = ctx.enter_context(tc.tile_pool(name="poolS", bufs=3))
        poolP = ctx.enter_context(tc.tile_pool(name="poolP", bufs=3))
        poolPT = ctx.enter_context(tc.tile_pool(name="poolPT", bufs=2))
        poolO = ctx.enter_context(tc.tile_pool(name="poolO", bufs=2))
        poolZ = ctx.enter_context(tc.tile_pool(name="poolZ", bufs=4))

        # ----- kv header scope: memT + split wkv, closed once kT/v are done --
        ctxKV = ctx.enter_context(ExitStack())
        poolKV = ctxKV.enter_context(tc.tile_pool(name="poolKV", bufs=1))
        ldM = ctxKV.enter_context(tc.tile_pool(name="ldM", bufs=2))
        memT = poolKV.tile([PDIM, NCC, T2], f32r, tag="memT")
        for mt in range(NTC):
            mtl = ldM.tile([PDIM, C], f32r, tag="ldM")
            nc.sync.dma_start(mtl[:], memory[mt * PDIM:(mt + 1) * PDIM, :].bitcast(f32r))
            pt = ps_t.tile([PDIM, C], f32, tag="pt")
            for cc in range(NCC):
                nc.tensor.transpose(
                    pt[:, cc * PDIM:(cc + 1) * PDIM].bitcast(f32r),
                    mtl[:, cc * PDIM:(cc + 1) * PDIM],
                    identr[:],
                )
            nc.vector.tensor_copy(
                memT[:, :, mt * PDIM:(mt + 1) * PDIM],
                pt[:].rearrange("p (cc t) -> p cc t", t=PDIM),
            )
        wk_sb = poolKV.tile([PDIM, NCC, C], f32r, tag="wk")
        nc.sync.dma_start(
            wk_sb[:], wkv[:, 0:C].rearrange("(cc p) j -> p cc j", p=PDIM)
        )
        wv_sb = poolKV.tile([PDIM, NCC, C], f32r, tag="wv")

        def emit_kT(hps):
            for hp in hps:
                ps = ps_s.tile([PDIM, T2], f32, tag="ps")
                for cc in range(NCC):
                    nc.tensor.matmul(
                        ps[:],
                        wk_sb[:, cc, hp * PDIM:(hp + 1) * PDIM],
                        memT[:, cc, :],
                        start=(cc == 0),
                        stop=(cc == NCC - 1),
                    )
                nc.vector.tensor_copy(kT[:, hp, :], ps[:])

        def emit_v():
            for tc_ in range(NTC):
                ps = ps_s.tile([PDIM, C], f32, tag="ps")
                for cc in range(NCC):
                    nc.tensor.matmul(
                        ps[:],
                        memT[:, cc, tc_ * PDIM:(tc_ + 1) * PDIM],
                        wv_sb[:, cc, :],
                        start=(cc == 0),
                        stop=(cc == NCC - 1),
                    )
                nc.vector.tensor_copy(v_sb[:, tc_, :], ps[:])

        emit_kT([0])
        nc.sync.dma_start(wq_sb[:], wq[:].rearrange("(cc p) j -> p cc j", p=PDIM))

        def emit_queryT(sp):
            queryT = poolQT.tile([PDIM, NCC, 512], f32r, tag="queryT")
            for jj in range(JJ):
                jt = sp * JJ + jj
                qt = poolIn.tile([PDIM, C], f32r, tag="ldA")
                nc.sync.dma_start(qt[:], query[jt * PDIM:(jt + 1) * PDIM, :].bitcast(f32r))
                pt = ps_t.tile([PDIM, C], f32, tag="pt")
                for cc in range(NCC):
                    nc.tensor.transpose(
                        pt[:, cc * PDIM:(cc + 1) * PDIM].bitcast(f32r),
                        qt[:, cc * PDIM:(cc + 1) * PDIM],
                        identr[:],
                    )
                nc.vector.tensor_copy(
                    queryT[:, :, jj * PDIM:(jj + 1) * PDIM],
                    pt[:].rearrange("p (cc t) -> p cc t", t=PDIM),
                )
            return queryT

        def emit_qT(queryT, qTs=None, hps=range(NCC)):
            if qTs is None:
                qTs = poolQT.tile([PDIM, NCC, 512], f32r, tag="qT")
            for hp in hps:
                ps = ps_s.tile([PDIM, 512], f32, tag="ps")
                for cc in range(NCC):
                    nc.tensor.matmul(
                        ps[:],
                        wq_sb[:, cc, hp * PDIM:(hp + 1) * PDIM],
                        queryT[:, cc, :],
                        start=(cc == 0),
                        stop=(cc == NCC - 1),
                    )
                nc.scalar.copy(qTs[:, hp, :], ps[:])
            return qTs

        def emit_outproj(sp):
            for jj in range(JJ):
                jt = sp * JJ + jj
                ps = ps_s.tile([PDIM, C], f32, tag="ps")
                for cc in range(NCC):
                    nc.tensor.matmul(
                        ps[:],
                        yT[:, cc, jt * PDIM:(jt + 1) * PDIM],
                        wo_sb[:, cc, :],
                        start=(cc == 0),
                        stop=(cc == NCC - 1),
                    )
                o_sb = poolO.tile([PDIM, C], f32)
                nc.scalar.copy(o_sb[:], ps[:])
                nc.sync.dma_start(out[jt * PDIM:(jt + 1) * PDIM, :], o_sb[:])

        def emit_L(sp):
            Ls = poolL.tile([PDIM, JJ, T2], f32r, tag="L")
            for jj in range(JJ):
                jt = sp * JJ + jj
                ptile = poolIn.tile([PDIM, T2], f32, tag="ldP")
                nc.sync.dma_start(ptile[:], prior[jt * PDIM:(jt + 1) * PDIM, :])
                nc.scalar.activation(
                    Ls[:, jj, :], ptile[:], AF.Ln, bias=eps_b[:]
                )
            return Ls

        pending_yT = [None]

        def flush_yT():
            if pending_yT[0] is not None:
                psy_, h_, sp_ = pending_yT[0]
                nc.vector.tensor_copy(
                    yT[(h_ % 2) * Dh:(h_ % 2 + 1) * Dh, h_ // 2,
                       sp_ * 512:(sp_ + 1) * 512],
                    psy_[:],
                )
                pending_yT[0] = None

        queryT_next = None
        qTs_next = None
        Ls_next = None
        queryT0 = emit_queryT(0)
        qTs = emit_qT(queryT0, hps=[0])
        Ls_cur = emit_L(0)
        nc.sync.dma_start(
            wv_sb[:], wkv[:, C:2 * C].rearrange("(cc p) j -> p cc j", p=PDIM)
        )
        nc.sync.dma_start(wo_sb[:], wo[:].rearrange("(cc p) j -> p cc j", p=PDIM))
        for sp in range(NSPAN):
            for h in range(H):
                if sp == 0 and h in (1, 3, 5):
                    hp = (h + 1) // 2
                    emit_kT([hp])
                    emit_qT(queryT0, qTs=qTs, hps=[hp])
                    if h == 5:
                        ctxKV.close()
                if h == 2 and sp + 1 < NSPAN:
                    Ls_next = emit_L(sp + 1)
                if h == 3 and sp + 1 < NSPAN:
                    queryT_next = emit_queryT(sp + 1)
                if h == 5 and sp + 1 < NSPAN:
                    qTs_next = emit_qT(queryT_next)
                flush_yT()
                if h == 1 and sp >= 1:
                    emit_outproj(sp - 1)
                hq = qTs[(h % 2) * Dh:(h % 2 + 1) * Dh, h // 2, :]
                hk = kT[(h % 2) * Dh:(h % 2 + 1) * Dh, h // 2, :]
                e_tiles = []
                zt = poolZ.tile([PDIM, JJ], f32, tag="z")
                rz = poolZ.tile([PDIM, JJ], f32, tag="rz")
                for jj in range(JJ):
                    jt = sp * JJ + jj
                    # score path
                    pss = ps_s.tile([PDIM, T2], f32, tag="ps")
                    nc.tensor.matmul(
                        pss[:],
                        hq[:, jj * PDIM:(jj + 1) * PDIM],
                        hk,
                        start=True,
                        stop=True,
                    )
                    s_sb = poolS.tile([PDIM, T2], f32)
                    if jj != 3:
                        nc.scalar.copy(s_sb[:], pss[:])
                    else:
                        nc.vector.tensor_copy(s_sb[:], pss[:])
                    nc.sync.dma_start(
                        score[h, jt * PDIM:(jt + 1) * PDIM, :], s_sb[:]
                    )
                    # exp path: recompute s, add L, exp (+Z via accum_out)
                    pse = ps_e.tile([PDIM, T2], f32, tag="pe")
                    nc.tensor.matmul(
                        pse[:],
                        hq[:, jj * PDIM:(jj + 1) * PDIM],
                        hk,
                        start=True,
                        stop=False,
                    )
                    nc.tensor.matmul(
                        pse[:],
                        identr[:],
                        Ls_cur[:, jj, :],
                        start=False,
                        stop=True,
                    )
                    e_sb = poolE.tile([PDIM, T2], f32)
                    nc.scalar.activation(
                        e_sb[:], pse[:], AF.Exp,
                        accum_out=zt[:, jj:jj + 1],
                    )
                    # per-tile reciprocal so the normalize of tile jj only
                    # waits on its own exp, not all four
                    nc.vector.reciprocal(rz[:, jj:jj + 1], zt[:, jj:jj + 1])
                    e_tiles.append(e_sb)
                if sp == 0 and h == 0:
                    emit_v()
                ptT = poolPT.tile([PDIM, NTC, 512], f32r)
                for jj in range(JJ):
                    jt = sp * JJ + jj
                    p_sb = poolP.tile([PDIM, T2], f32r)
                    nc.vector.tensor_scalar_mul(
                        p_sb[:], e_tiles[jj][:], rz[:, jj:jj + 1]
                    )
                    nc.sync.dma_start(
                        prob[h, jt * PDIM:(jt + 1) * PDIM, :].bitcast(f32r),
                        p_sb[:],
                    )
                    pst = ps_t.tile([PDIM, T2], f32, tag="pt")
                    for tc_ in range(NTC):
                        nc.tensor.transpose(
                            pst[:, tc_ * PDIM:(tc_ + 1) * PDIM].bitcast(f32r),
                            p_sb[:, tc_ * PDIM:(tc_ + 1) * PDIM],
                            identr[:],
                        )
                    nc.vector.tensor_copy(
                        ptT[:, :, jj * PDIM:(jj + 1) * PDIM],
                        pst[:].rearrange("p (tc t) -> p tc t", t=PDIM),
                    )
                # y^T_h[span] = v_h^T @ P^T
                psy = ps_y.tile([Dh, 512], f32, tag="py")
                for tc_ in range(NTC):
                    nc.tensor.matmul(
                        psy[:],
                        v_sb[:, tc_, h * Dh:(h + 1) * Dh],
                        ptT[:, tc_, :],
                        start=(tc_ == 0),
                        stop=(tc_ == NTC - 1),
                    )
                pending_yT[0] = (psy, h, sp)

            if sp + 1 < NSPAN:
                qTs = qTs_next
                Ls_cur = Ls_next
        flush_yT()
        emit_outproj(NSPAN - 1)

    nc.compile()
    bacc.get_activation_tables = _orig_tabs
    return nc


_NC_CACHE = {}


def kernel(query, query_mask, memory, memory_mask, attn_prior, Wq, Wkv, Wo):
    from concourse.bass_utils import run_bass_kernel_spmd

    query = np.asarray(query, dtype=np.float32)
    memory = np.asarray(memory, dtype=np.float32)
    attn_prior = np.asarray(attn_prior, dtype=np.float32)
    scale = np.float32(Dh ** -0.5)
    wq_s = np.ascontiguousarray(np.asarray(Wq, dtype=np.float32) * scale)
    wkv_h = np.ascontiguousarray(np.asarray(Wkv, dtype=np.float32))
    wo_h = np.ascontiguousarray(np.asarray(Wo, dtype=np.float32))

    if "nc" not in _NC_CACHE:
        _NC_CACHE["nc"] = _build_nc()
    nc = _NC_CACHE["nc"]

    in_maps = [
        {
            "query": np.ascontiguousarray(query[b]),
            "memory": np.ascontiguousarray(memory[b]),
            "prior": np.ascontiguousarray(attn_prior[b]),
            "wq": wq_s,
            "wkv": wkv_h,
            "wo": wo_h,
        }
        for b in range(B)
    ]
    res = run_bass_kernel_spmd(nc, in_maps, core_ids=list(range(B)))
    _NC_CACHE["last_result"] = res
    out = np.stack([r["out"] for r in res.results])
    prob = np.stack([r["prob"] for r in res.results])
    score = np.stack([r["score"] for r in res.results])
    return out, prob, score


# revision 57
# speedup vs baseline: 1.2282x; 1.0165x over previous
"""Trainium2 Bass kernel for prior-fused cross-attention.

Math (per batch b, head h):
  q  = query @ (Wq * Dh^-0.5)            # scale folded into Wq on host
  k,v = split(memory @ Wkv)
  s  = q_h @ k_h^T                       # == attn_score output (mask all-ones)
  L  = ln(prior + eps)
  P  = softmax(s + L)                    # == softmax(log_softmax(s) + L)
  y  = P_h @ v_h ;  out = y @ Wo

Layout strategy (per core = one batch element):
  - matmuls in float32r (1 cyc/row at N>=512); every matmul operand tile is
    declared float32r so its producer (copy/activation/DMA) does the rounding
    the BIR verifier demands.
  - fully pipelined over 4 t1-spans of 512: per span, transpose query on PE,
    project qT, ln(prior), then per head: s into one PSUM bank (copied out for
    the attn_score output) and s+L into another (identity-weight matmul
    accumulate) so exp never waits on the score copy/DMA; exp on ACT with
    accum_out giving the softmax denominator Z for free; P normalized with a
    per-partition tensor_scalar (split gpsimd/DVE); P transposed on PE to feed
    the P@v matmul; y kept transposed [c-part, t1-free] so the Wo matmul
    consumes it as lhsT; out-projection folded into each span.
"""

import numpy as np

B, T1, T2, C, H = 8, 2048, 512, 512, 8
Dh = C // H
EPS = 1e-8
PDIM = 128
NT1 = T1 // PDIM      # 16 t1 tiles
NSPAN = T1 // 512     # 4 t1 spans of 512
NCC = C // PDIM       # 4 contraction chunks
NTC = T2 // PDIM      # 4 t2 chunks
JJ = 512 // PDIM      # 4 t1-tiles per span


def _build_nc():
    from contextlib import ExitStack

    import concourse.bacc as bacc
    import concourse.mybir as mybir
    import concourse.tile as tile
    from concourse.masks import make_identity

    f32 = mybir.dt.float32
    f32r = mybir.dt.float32r
    AF = mybir.ActivationFunctionType

    # Prefer the act-function set holding Ln+Exp+Copy together so per-span
    # interleaving of ln/exp/copy needs a single table load.
    import concourse.hw_specs as _hw
    _orig_tabs = _hw.get_activation_tables

    def _tabs(arch, *a, **k):
        # Keep dict order (act_func_set_id = insertion index!) but strip
        # Ln/Exp from every other set so the selector lands on the combined
        # Ln+Exp+Copy set, minimizing runtime table reloads.
        import concourse.mybir as _mb
        tabs = dict(_orig_tabs(arch, *a, **k))
        strip = {_mb.ActivationFunctionType.Ln, _mb.ActivationFunctionType.Exp}
        return {
            name: (fns if name == 'natural_log_exp_and_others'
                   else set(fns) - strip)
            for name, fns in tabs.items()
        }

    bacc.get_activation_tables = _tabs

    nc = bacc.Bacc()
    query = nc.dram_tensor("query", [T1, C], f32, kind="ExternalInput")
    memory = nc.dram_tensor("memory", [T2, C], f32, kind="ExternalInput")
    prior = nc.dram_tensor("prior", [T1, T2], f32, kind="ExternalInput")
    wq = nc.dram_tensor("wq", [C, C], f32r, kind="ExternalInput")  # pre-scaled
    wkv = nc.dram_tensor("wkv", [C, 2 * C], f32r, kind="ExternalInput")
    wo = nc.dram_tensor("wo", [C, C], f32r, kind="ExternalInput")
    out = nc.dram_tensor("out", [T1, C], f32, kind="ExternalOutput")
    prob = nc.dram_tensor("prob", [H, T1, T2], f32, kind="ExternalOutput")
    score = nc.dram_tensor("score", [H, T1, T2], f32, kind="ExternalOutput")

    with tile.TileContext(nc) as tc, ExitStack() as ctx:
        singles = ctx.enter_context(tc.tile_pool(name="singles", bufs=1))
        ps_s = ctx.enter_context(tc.tile_pool(name="ps_s", bufs=3, space="PSUM"))
        ps_e = ctx.enter_context(tc.tile_pool(name="ps_e", bufs=2, space="PSUM"))
        ps_t = ctx.enter_context(tc.tile_pool(name="ps_t", bufs=2, space="PSUM"))
        ps_y = ctx.enter_context(tc.tile_pool(name="ps_y", bufs=1, space="PSUM"))

        ident = singles.tile([PDIM, PDIM], f32)
        make_identity(nc, ident[:])
        identr = singles.tile([PDIM, PDIM], f32r)
        nc.vector.tensor_copy(identr[:], ident[:])
        eps_b = singles.tile([PDIM, 1], f32)
        nc.vector.memset(eps_b[:], EPS)

        kT = singles.tile([PDIM, NCC, T2], f32r)
        v_sb = singles.tile([PDIM, NTC, C], f32r)
        wq_sb = singles.tile([PDIM, NCC, C], f32r)
        wo_sb = singles.tile([PDIM, NCC, C], f32r)

        # main pools first so the kv-header scope can close mid-stream (LIFO)
        poolIn = ctx.enter_context(tc.tile_pool(name="poolIn", bufs=3))
        poolQT = ctx.enter_context(tc.tile_pool(name="poolQT", bufs=2))
        poolL = ctx.enter_context(tc.tile_pool(name="poolL", bufs=2))
        poolB = ctx.enter_context(tc.tile_pool(name="poolB", bufs=1))
        yT = poolB.tile([PDIM, NCC, T1], f32r, tag="yT")
        poolE = ctx.enter_context(tc.tile_pool(name="poolE", bufs=8))
        poolS = ctx.enter_context(tc.tile_pool(name="poolS", bufs=3))
        poolP = ctx.enter_context(tc.tile_pool(name="poolP", bufs=6))
        poolPT = ctx.enter_context(tc.tile_pool(name="poolPT", bufs=2))
        poolO = ctx.enter_context(tc.tile_pool(name="poolO", bufs=2))
        poolZ = ctx.enter_context(tc.tile_pool(name="poolZ", bufs=4))

        # ----- kv header scope: memT + split wkv, closed once kT/v are done --
        ctxKV = ctx.enter_context(ExitStack())
        poolKV = ctxKV.enter_context(tc.tile_pool(name="poolKV", bufs=1))
        ldM = ctxKV.enter_context(tc.tile_pool(name="ldM", bufs=2))
        memT = poolKV.tile([PDIM, NCC, T2], f32r, tag="memT")
        for mt in range(NTC):
            mtl = ldM.tile([PDIM, C], f32r, tag="ldM")
            nc.sync.dma_start(mtl[:], memory[mt * PDIM:(mt + 1) * PDIM, :].bitcast(f32r))
            pt = ps_t.tile([PDIM, C], f32, tag="pt")
            for cc in range(NCC):
                nc.tensor.transpose(
                    pt[:, cc * PDIM:(cc + 1) * PDIM].bitcast(f32r),
                    mtl[:, cc * PDIM:(cc + 1) * PDIM],
                    identr[:],
                )
            nc.vector.tensor_copy(
                memT[:, :, mt * PDIM:(mt + 1) * PDIM],
                pt[:].rearrange("p (cc t) -> p cc t", t=PDIM),
            )
        wk_sb = poolKV.tile([PDIM, NCC, C], f32r, tag="wk")
        nc.sync.dma_start(
            wk_sb[:], wkv[:, 0:C].rearrange("(cc p) j -> p cc j", p=PDIM)
        )
        wv_sb = poolKV.tile([PDIM, NCC, C], f32r, tag="wv")

        def emit_kT(hps):
            for hp in hps:
                ps = ps_s.tile([PDIM, T2], f32, tag="ps")
                for cc in range(NCC):
                    nc.tensor.matmul(
                        ps[:],
                        wk_sb[:, cc, hp * PDIM:(hp + 1) * PDIM],
                        memT[:, cc, :],
                        start=(cc == 0),
                        stop=(cc == NCC - 1),
                    )
                nc.vector.tensor_copy(kT[:, hp, :], ps[:])

        def emit_v():
            for tc_ in range(NTC):
                ps = ps_s.tile([PDIM, C], f32, tag="ps")
                for cc in range(NCC):
                    nc.tensor.matmul(
                        ps[:],
                        memT[:, cc, tc_ * PDIM:(tc_ + 1) * PDIM],
                        wv_sb[:, cc, :],
                        start=(cc == 0),
                        stop=(cc == NCC - 1),
                    )
                nc.vector.tensor_copy(v_sb[:, tc_, :], ps[:])

        emit_kT([0])
        nc.sync.dma_start(wq_sb[:], wq[:].rearrange("(cc p) j -> p cc j", p=PDIM))

        def emit_queryT(sp):
            queryT = poolQT.tile([PDIM, NCC, 512], f32r, tag="queryT")
            for jj in range(JJ):
                jt = sp * JJ + jj
                qt = poolIn.tile([PDIM, C], f32r, tag="ldA")
                nc.sync.dma_start(qt[:], query[jt * PDIM:(jt + 1) * PDIM, :].bitcast(f32r))
                pt = ps_t.tile([PDIM, C], f32, tag="pt")
                for cc in range(NCC):
                    nc.tensor.transpose(
                        pt[:, cc * PDIM:(cc + 1) * PDIM].bitcast(f32r),
                        qt[:, cc * PDIM:(cc + 1) * PDIM],
                        identr[:],
                    )
                nc.vector.tensor_copy(
                    queryT[:, :, jj * PDIM:(jj + 1) * PDIM],
                    pt[:].rearrange("p (cc t) -> p cc t", t=PDIM),
                )
            return queryT

        def emit_qT(queryT, qTs=None, hps=range(NCC)):
            if qTs is None:
                qTs = poolQT.tile([PDIM, NCC, 512], f32r, tag="qT")
            for hp in hps:
                ps = ps_s.tile([PDIM, 512], f32, tag="ps")
                for cc in range(NCC):
                    nc.tensor.matmul(
                        ps[:],
                        wq_sb[:, cc, hp * PDIM:(hp + 1) * PDIM],
                        queryT[:, cc, :],
                        start=(cc == 0),
                        stop=(cc == NCC - 1),
                    )
                nc.scalar.copy(qTs[:, hp, :], ps[:])
            return qTs

        def emit_outproj(sp):
            for jj in range(JJ):
                jt = sp * JJ + jj
                ps = ps_s.tile([PDIM, C], f32, tag="ps")
                for cc in range(NCC):
                    nc.tensor.matmul(
                        ps[:],
                        yT[:, cc, jt * PDIM:(jt + 1) * PDIM],
                        wo_sb[:, cc, :],
                        start=(cc == 0),
                        stop=(cc == NCC - 1),
                    )
                o_sb = poolO.tile([PDIM, C], f32)
                nc.scalar.copy(o_sb[:], ps[:])
                nc.sync.dma_start(out[jt * PDIM:(jt + 1) * PDIM, :], o_sb[:])

        def emit_L(sp):
            Ls = poolL.tile([PDIM, JJ, T2], f32r, tag="L")
            for jj in range(JJ):
                jt = sp * JJ + jj
                ptile = poolIn.tile([PDIM, T2], f32, tag="ldP")
                nc.sync.dma_start(ptile[:], prior[jt * PDIM:(jt + 1) * PDIM, :])
                nc.scalar.activation(
                    Ls[:, jj, :], ptile[:], AF.Ln, bias=eps_b[:]
                )
            return Ls

        pending_probs = []

        def emit_pending_prob(n):
            for _ in range(min(n, len(pending_probs))):
                p_sb_, h_, jt_ = pending_probs.pop(0)
                nc.sync.dma_start(
                    prob[h_, jt_ * PDIM:(jt_ + 1) * PDIM, :].bitcast(f32r),
                    p_sb_[:],
                )

        pending_yT = [None]

        def flush_yT():
            if pending_yT[0] is not None:
                psy_, h_, sp_ = pending_yT[0]
                nc.vector.tensor_copy(
                    yT[(h_ % 2) * Dh:(h_ % 2 + 1) * Dh, h_ // 2,
                       sp_ * 512:(sp_ + 1) * 512],
                    psy_[:],
                )
                pending_yT[0] = None

        queryT_next = None
        qTs_next = None
        Ls_next = None
        queryT0 = emit_queryT(0)
        qTs = emit_qT(queryT0, hps=[0])
        Ls_cur = emit_L(0)
        nc.sync.dma_start(
            wv_sb[:], wkv[:, C:2 * C].rearrange("(cc p) j -> p cc j", p=PDIM)
        )
        nc.sync.dma_start(wo_sb[:], wo[:].rearrange("(cc p) j -> p cc j", p=PDIM))
        for sp in range(NSPAN):
            for h in range(H):
                if sp == 0 and h in (1, 3, 5):
                    hp = (h + 1) // 2
                    emit_kT([hp])
                    emit_qT(queryT0, qTs=qTs, hps=[hp])
                    if h == 5:
                        ctxKV.close()
                if h == 2 and sp + 1 < NSPAN:
                    Ls_next = emit_L(sp + 1)
                if h == 3 and sp + 1 < NSPAN:
                    queryT_next = emit_queryT(sp + 1)
                if h == 5 and sp + 1 < NSPAN:
                    qTs_next = emit_qT(queryT_next)
                flush_yT()
                if h == 1 and sp >= 1:
                    emit_outproj(sp - 1)
                hq = qTs[(h % 2) * Dh:(h % 2 + 1) * Dh, h // 2, :]
                hk = kT[(h % 2) * Dh:(h % 2 + 1) * Dh, h // 2, :]
                e_tiles = []
                zt = poolZ.tile([PDIM, JJ], f32, tag="z")
                rz = poolZ.tile([PDIM, JJ], f32, tag="rz")
                for jj in range(JJ):
                    jt = sp * JJ + jj
                    # score path
                    pss = ps_s.tile([PDIM, T2], f32, tag="ps")
                    nc.tensor.matmul(
                        pss[:],
                        hq[:, jj * PDIM:(jj + 1) * PDIM],
                        hk,
                        start=True,
                        stop=True,
                    )
                    s_sb = poolS.tile([PDIM, T2], f32)
                    if jj != 3:
                        nc.scalar.copy(s_sb[:], pss[:])
                    else:
                        nc.vector.tensor_copy(s_sb[:], pss[:])
                    nc.sync.dma_start(
                        score[h, jt * PDIM:(jt + 1) * PDIM, :], s_sb[:]
                    )
                    emit_pending_prob(1)
                    # exp path: recompute s, add L, exp (+Z via accum_out)
                    pse = ps_e.tile([PDIM, T2], f32, tag="pe")
                    nc.tensor.matmul(
                        pse[:],
                        hq[:, jj * PDIM:(jj + 1) * PDIM],
                        hk,
                        start=True,
                        stop=False,
                    )
                    nc.tensor.matmul(
                        pse[:],
                        identr[:],
                        Ls_cur[:, jj, :],
                        start=False,
                        stop=True,
                    )
                    e_sb = poolE.tile([PDIM, T2], f32)
                    nc.scalar.activation(
                        e_sb[:], pse[:], AF.Exp,
                        accum_out=zt[:, jj:jj + 1],
                    )
                    # per-tile reciprocal so the normalize of tile jj only
                    # waits on its own exp, not all four
                    nc.vector.reciprocal(rz[:, jj:jj + 1], zt[:, jj:jj + 1])
                    e_tiles.append(e_sb)
                if sp == 0 and h == 0:
                    emit_v()
                ptT = poolPT.tile([PDIM, NTC, 512], f32r)
                for jj in range(JJ):
                    jt = sp * JJ + jj
                    p_sb = poolP.tile([PDIM, T2], f32r)
                    nc.vector.tensor_scalar_mul(
                        p_sb[:], e_tiles[jj][:], rz[:, jj:jj + 1]
                    )
                    pending_probs.append((p_sb, h, jt))
                    pst = ps_t.tile([PDIM, T2], f32, tag="pt")
                    for tc_ in range(NTC):
                        nc.tensor.transpose(
                            pst[:, tc_ * PDIM:(tc_ + 1) * PDIM].bitcast(f32r),
                            p_sb[:, tc_ * PDIM:(tc_ + 1) * PDIM],
                            identr[:],
                        )
                    nc.vector.tensor_copy(
                        ptT[:, :, jj * PDIM:(jj + 1) * PDIM],
                        pst[:].rearrange("p (tc t) -> p tc t", t=PDIM),
                    )
                # y^T_h[span] = v_h^T @ P^T
                psy = ps_y.tile([Dh, 512], f32, tag="py")
                for tc_ in range(NTC):
                    nc.tensor.matmul(
                        psy[:],
                        v_sb[:, tc_, h * Dh:(h + 1) * Dh],
                        ptT[:, tc_, :],
                        start=(tc_ == 0),
                        stop=(tc_ == NTC - 1),
                    )
                pending_yT[0] = (psy, h, sp)

            if sp + 1 < NSPAN:
                qTs = qTs_next
                Ls_cur = Ls_next
        emit_pending_prob(len(pending_probs))
        flush_yT()
        emit_outproj(NSPAN - 1)

    nc.compile()
    bacc.get_activation_tables = _orig_tabs
    return nc


_NC_CACHE = {}


def kernel(query, query_mask, memory, memory_mask, attn_prior, Wq, Wkv, Wo):
    from concourse.bass_utils import run_bass_kernel_spmd

    query = np.asarray(query, dtype=np.float32)
    memory = np.asarray(memory, dtype=np.float32)
    attn_prior = np.asarray(attn_prior, dtype=np.float32)
    scale = np.float32(Dh ** -0.5)
    wq_s = np.ascontiguousarray(np.asarray(Wq, dtype=np.float32) * scale)
    wkv_h = np.ascontiguousarray(np.asarray(Wkv, dtype=np.float32))
    wo_h = np.ascontiguousarray(np.asarray(Wo, dtype=np.float32))

    if "nc" not in _NC_CACHE:
        _NC_CACHE["nc"] = _build_nc()
    nc = _NC_CACHE["nc"]

    in_maps = [
        {
            "query": np.ascontiguousarray(query[b]),
            "memory": np.ascontiguousarray(memory[b]),
            "prior": np.ascontiguousarray(attn_prior[b]),
            "wq": wq_s,
            "wkv": wkv_h,
            "wo": wo_h,
        }
        for b in range(B)
    ]
    res = run_bass_kernel_spmd(nc, in_maps, core_ids=list(range(B)))
    _NC_CACHE["last_result"] = res
    out = np.stack([r["out"] for r in res.results])
    prob = np.stack([r["prob"] for r in res.results])
    score = np.stack([r["score"] for r in res.results])
    return out, prob, score


# revision 62
# speedup vs baseline: 1.2297x; 1.0012x over previous
"""Trainium2 Bass kernel for prior-fused cross-attention.

Math (per batch b, head h):
  q  = query @ (Wq * Dh^-0.5)            # scale folded into Wq on host
  k,v = split(memory @ Wkv)
  s  = q_h @ k_h^T                       # == attn_score output (mask all-ones)
  L  = ln(prior + eps)
  P  = softmax(s + L)                    # == softmax(log_softmax(s) + L)
  y  = P_h @ v_h ;  out = y @ Wo

Layout strategy (per core = one batch element):
  - matmuls in float32r (1 cyc/row at N>=512); every matmul operand tile is
    declared float32r so its producer (copy/activation/DMA) does the rounding
    the BIR verifier demands.
  - fully pipelined over 4 t1-spans of 512: per span, transpose query on PE,
    project qT, ln(prior), then per head: s into one PSUM bank (copied out for
    the attn_score output) and s+L into another (identity-weight matmul
    accumulate) so exp never waits on the score copy/DMA; exp on ACT with
    accum_out giving the softmax denominator Z for free; P normalized with a
    per-partition tensor_scalar (split gpsimd/DVE); P transposed on PE to feed
    the P@v matmul; y kept transposed [c-part, t1-free] so the Wo matmul
    consumes it as lhsT; out-projection folded into each span.
"""

import numpy as np

B, T1, T2, C, H = 8, 2048, 512, 512, 8
Dh = C // H
EPS = 1e-8
PDIM = 128
NT1 = T1 // PDIM      # 16 t1 tiles
NSPAN = T1 // 512     # 4 t1 spans of 512
NCC = C // PDIM       # 4 contraction chunks
NTC = T2 // PDIM      # 4 t2 chunks
JJ = 512 // PDIM      # 4 t1-tiles per span


def _build_nc():
    from contextlib import ExitStack

    import concourse.bacc as bacc
    import concourse.mybir as mybir
    import concourse.tile as tile
    from concourse.masks import make_identity

    f32 = mybir.dt.float32
    f32r = mybir.dt.float32r
    AF = mybir.ActivationFunctionType

    # Prefer the act-function set holding Ln+Exp+Copy together so per-span
    # interleaving of ln/exp/copy needs a single table load.
    import concourse.hw_specs as _hw
    _orig_tabs = _hw.get_activation_tables

    def _tabs(arch, *a, **k):
        # Keep dict order (act_func_set_id = insertion index!) but strip
        # Ln/Exp from every other set so the selector lands on the combined
        # Ln+Exp+Copy set, minimizing runtime table reloads.
        import concourse.mybir as _mb
        tabs = dict(_orig_tabs(arch, *a, **k))
        strip = {_mb.ActivationFunctionType.Ln, _mb.ActivationFunctionType.Exp}
        return {
            name: (fns if name == 'natural_log_exp_and_others'
                   else set(fns) - strip)
            for name, fns in tabs.items()
        }

    bacc.get_activation_tables = _tabs

    nc = bacc.Bacc()
    query = nc.dram_tensor("query", [T1, C], f32, kind="ExternalInput")
    memory = nc.dram_tensor("memory", [T2, C], f32, kind="ExternalInput")
    prior = nc.dram_tensor("prior", [T1, T2], f32, kind="ExternalInput")
    wq = nc.dram_tensor("wq", [C, C], f32r, kind="ExternalInput")  # pre-scaled
    wkv = nc.dram_tensor("wkv", [C, 2 * C], f32r, kind="ExternalInput")
    wo = nc.dram_tensor("wo", [C, C], f32r, kind="ExternalInput")
    out = nc.dram_tensor("out", [T1, C], f32, kind="ExternalOutput")
    prob = nc.dram_tensor("prob", [H, T1, T2], f32, kind="ExternalOutput")
    score = nc.dram_tensor("score", [H, T1, T2], f32, kind="ExternalOutput")

    with tile.TileContext(nc) as tc, ExitStack() as ctx:
        singles = ctx.enter_context(tc.tile_pool(name="singles", bufs=1))
        ps_s = ctx.enter_context(tc.tile_pool(name="ps_s", bufs=3, space="PSUM"))
        ps_e = ctx.enter_context(tc.tile_pool(name="ps_e", bufs=2, space="PSUM"))
        ps_t = ctx.enter_context(tc.tile_pool(name="ps_t", bufs=2, space="PSUM"))
        ps_y = ctx.enter_context(tc.tile_pool(name="ps_y", bufs=1, space="PSUM"))

        ident = singles.tile([PDIM, PDIM], f32)
        make_identity(nc, ident[:])
        identr = singles.tile([PDIM, PDIM], f32r)
        nc.vector.tensor_copy(identr[:], ident[:])
        eps_b = singles.tile([PDIM, 1], f32)
        nc.vector.memset(eps_b[:], EPS)

        kT = singles.tile([PDIM, NCC, T2], f32r)
        v_sb = singles.tile([PDIM, NTC, C], f32r)
        wq_sb = singles.tile([PDIM, NCC, C], f32r)
        wo_sb = singles.tile([PDIM, NCC, C], f32r)

        # main pools first so the kv-header scope can close mid-stream (LIFO)
        poolIn = ctx.enter_context(tc.tile_pool(name="poolIn", bufs=3))
        poolQT = ctx.enter_context(tc.tile_pool(name="poolQT", bufs=2))
        poolL = ctx.enter_context(tc.tile_pool(name="poolL", bufs=2))
        poolB = ctx.enter_context(tc.tile_pool(name="poolB", bufs=1))
        yT = poolB.tile([PDIM, NCC, T1], f32r, tag="yT")
        poolE = ctx.enter_context(tc.tile_pool(name="poolE", bufs=8))
        poolS = ctx.enter_context(tc.tile_pool(name="poolS", bufs=3))
        poolP = ctx.enter_context(tc.tile_pool(name="poolP", bufs=6))
        poolPT = ctx.enter_context(tc.tile_pool(name="poolPT", bufs=2))
        poolO = ctx.enter_context(tc.tile_pool(name="poolO", bufs=2))
        poolZ = ctx.enter_context(tc.tile_pool(name="poolZ", bufs=4))

        # ----- kv header scope: memT + split wkv, closed once kT/v are done --
        ctxKV = ctx.enter_context(ExitStack())
        poolKV = ctxKV.enter_context(tc.tile_pool(name="poolKV", bufs=1))
        ldM = ctxKV.enter_context(tc.tile_pool(name="ldM", bufs=2))
        memT = poolKV.tile([PDIM, NCC, T2], f32r, tag="memT")
        for mt in range(NTC):
            mtl = ldM.tile([PDIM, C], f32r, tag="ldM")
            nc.sync.dma_start(mtl[:], memory[mt * PDIM:(mt + 1) * PDIM, :].bitcast(f32r))
            pt = ps_t.tile([PDIM, C], f32, tag="pt")
            for cc in range(NCC):
                nc.tensor.transpose(
                    pt[:, cc * PDIM:(cc + 1) * PDIM].bitcast(f32r),
                    mtl[:, cc * PDIM:(cc + 1) * PDIM],
                    identr[:],
                )
            nc.vector.tensor_copy(
                memT[:, :, mt * PDIM:(mt + 1) * PDIM],
                pt[:].rearrange("p (cc t) -> p cc t", t=PDIM),
            )
        wk_sb = poolKV.tile([PDIM, NCC, C], f32r, tag="wk")
        nc.sync.dma_start(
            wk_sb[:], wkv[:, 0:C].rearrange("(cc p) j -> p cc j", p=PDIM)
        )
        wv_sb = poolKV.tile([PDIM, NCC, C], f32r, tag="wv")

        def emit_kT(hps):
            for hp in hps:
                ps = ps_s.tile([PDIM, T2], f32, tag="ps")
                for cc in range(NCC):
                    nc.tensor.matmul(
                        ps[:],
                        wk_sb[:, cc, hp * PDIM:(hp + 1) * PDIM],
                        memT[:, cc, :],
                        start=(cc == 0),
                        stop=(cc == NCC - 1),
                    )
                nc.vector.tensor_copy(kT[:, hp, :], ps[:])

        def emit_v():
            for tc_ in range(NTC):
                ps = ps_s.tile([PDIM, C], f32, tag="ps")
                for cc in range(NCC):
                    nc.tensor.matmul(
                        ps[:],
                        memT[:, cc, tc_ * PDIM:(tc_ + 1) * PDIM],
                        wv_sb[:, cc, :],
                        start=(cc == 0),
                        stop=(cc == NCC - 1),
                    )
                nc.vector.tensor_copy(v_sb[:, tc_, :], ps[:])

        emit_kT([0])
        nc.sync.dma_start(wq_sb[:], wq[:].rearrange("(cc p) j -> p cc j", p=PDIM))

        def emit_queryT(sp):
            queryT = poolQT.tile([PDIM, NCC, 512], f32r, tag="queryT")
            for jj in range(JJ):
                jt = sp * JJ + jj
                qt = poolIn.tile([PDIM, C], f32r, tag="ldA")
                nc.sync.dma_start(qt[:], query[jt * PDIM:(jt + 1) * PDIM, :].bitcast(f32r))
                pt = ps_t.tile([PDIM, C], f32, tag="pt")
                for cc in range(NCC):
                    nc.tensor.transpose(
                        pt[:, cc * PDIM:(cc + 1) * PDIM].bitcast(f32r),
                        qt[:, cc * PDIM:(cc + 1) * PDIM],
                        identr[:],
                    )
                nc.vector.tensor_copy(
                    queryT[:, :, jj * PDIM:(jj + 1) * PDIM],
                    pt[:].rearrange("p (cc t) -> p cc t", t=PDIM),
                )
            return queryT

        def emit_qT(queryT, qTs=None, hps=range(NCC)):
            if qTs is None:
                qTs = poolQT.tile([PDIM, NCC, 512], f32r, tag="qT")
            for hp in hps:
                ps = ps_s.tile([PDIM, 512], f32, tag="ps")
                for cc in range(NCC):
                    nc.tensor.matmul(
                        ps[:],
                        wq_sb[:, cc, hp * PDIM:(hp + 1) * PDIM],
                        queryT[:, cc, :],
                        start=(cc == 0),
                        stop=(cc == NCC - 1),
                    )
                nc.scalar.copy(qTs[:, hp, :], ps[:])
            return qTs

        def emit_outproj(sp):
            for jj in range(JJ):
                jt = sp * JJ + jj
                ps = ps_s.tile([PDIM, C], f32, tag="ps")
                for cc in range(NCC):
                    nc.tensor.matmul(
                        ps[:],
                        yT[:, cc, jt * PDIM:(jt + 1) * PDIM],
                        wo_sb[:, cc, :],
                        start=(cc == 0),
                        stop=(cc == NCC - 1),
                    )
                o_sb = poolO.tile([PDIM, C], f32)
                nc.scalar.copy(o_sb[:], ps[:])
                nc.sync.dma_start(out[jt * PDIM:(jt + 1) * PDIM, :], o_sb[:])

        def emit_L(sp):
            Ls = poolL.tile([PDIM, JJ, T2], f32r, tag="L")
            for jj in range(JJ):
                jt = sp * JJ + jj
                ptile = poolIn.tile([PDIM, T2], f32, tag="ldP")
                nc.sync.dma_start(ptile[:], prior[jt * PDIM:(jt + 1) * PDIM, :])
                nc.scalar.activation(
                    Ls[:, jj, :], ptile[:], AF.Ln, bias=eps_b[:]
                )
            return Ls

        pending_probs = []

        def emit_pending_prob(n):
            for _ in range(min(n, len(pending_probs))):
                p_sb_, h_, jt_ = pending_probs.pop(0)
                nc.sync.dma_start(
                    prob[h_, jt_ * PDIM:(jt_ + 1) * PDIM, :].bitcast(f32r),
                    p_sb_[:],
                )

        pending_yT = [None]

        def flush_yT():
            if pending_yT[0] is not None:
                psy_, h_, sp_ = pending_yT[0]
                nc.vector.tensor_copy(
                    yT[(h_ % 2) * Dh:(h_ % 2 + 1) * Dh, h_ // 2,
                       sp_ * 512:(sp_ + 1) * 512],
                    psy_[:],
                )
                pending_yT[0] = None

        queryT_next = None
        qTs_next = None
        Ls_next = None
        queryT0 = emit_queryT(0)
        qTs = emit_qT(queryT0, hps=[0])
        Ls_cur = emit_L(0)
        nc.sync.dma_start(
            wv_sb[:], wkv[:, C:2 * C].rearrange("(cc p) j -> p cc j", p=PDIM)
        )
        nc.sync.dma_start(wo_sb[:], wo[:].rearrange("(cc p) j -> p cc j", p=PDIM))
        for sp in range(NSPAN):
            for h in range(H):
                if sp == 0 and h in (1, 3, 5):
                    hp = (h + 1) // 2
                    emit_kT([hp])
                    emit_qT(queryT0, qTs=qTs, hps=[hp])
                    if h == 5:
                        ctxKV.close()
                if h == 2 and sp + 1 < NSPAN:
                    Ls_next = emit_L(sp + 1)
                if h == 3 and sp + 1 < NSPAN:
                    queryT_next = emit_queryT(sp + 1)
                if h == 5 and sp + 1 < NSPAN:
                    qTs_next = emit_qT(queryT_next)
                flush_yT()
                if h == 1 and sp >= 1:
                    emit_outproj(sp - 1)
                hq = qTs[(h % 2) * Dh:(h % 2 + 1) * Dh, h // 2, :]
                hk = kT[(h % 2) * Dh:(h % 2 + 1) * Dh, h // 2, :]
                e_tiles = []
                zt = poolZ.tile([PDIM, JJ], f32, tag="z")
                rz = poolZ.tile([PDIM, JJ], f32, tag="rz")
                for jj in range(JJ):
                    jt = sp * JJ + jj
                    # score path
                    pss = ps_s.tile([PDIM, T2], f32, tag="ps")
                    nc.tensor.matmul(
                        pss[:],
                        hq[:, jj * PDIM:(jj + 1) * PDIM],
                        hk,
                        start=True,
                        stop=True,
                    )
                    s_sb = poolS.tile([PDIM, T2], f32)
                    if jj != 0:
                        nc.scalar.copy(s_sb[:], pss[:])
                    else:
                        nc.vector.tensor_copy(s_sb[:], pss[:])
                    nc.sync.dma_start(
                        score[h, jt * PDIM:(jt + 1) * PDIM, :], s_sb[:]
                    )
                    emit_pending_prob(1)
                    # exp path: recompute s, add L, exp (+Z via accum_out)
                    pse = ps_e.tile([PDIM, T2], f32, tag="pe")
                    nc.tensor.matmul(
                        pse[:],
                        hq[:, jj * PDIM:(jj + 1) * PDIM],
                        hk,
                        start=True,
                        stop=False,
                    )
                    nc.tensor.matmul(
                        pse[:],
                        identr[:],
                        Ls_cur[:, jj, :],
                        start=False,
                        stop=True,
                    )
                    e_sb = poolE.tile([PDIM, T2], f32)
                    nc.scalar.activation(
                        e_sb[:], pse[:], AF.Exp,
                        accum_out=zt[:, jj:jj + 1],
                    )
                    # per-tile reciprocal so the normalize of tile jj only
                    # waits on its own exp, not all four
                    nc.vector.reciprocal(rz[:, jj:jj + 1], zt[:, jj:jj + 1])
                    e_tiles.append(e_sb)
                if sp == 0 and h == 0:
                    emit_v()
                ptT = poolPT.tile([PDIM, NTC, 512], f32r)
                for jj in range(JJ):
                    jt = sp * JJ + jj
                    p_sb = poolP.tile([PDIM, T2], f32r)
                    nc.vector.tensor_scalar_mul(
                        p_sb[:], e_tiles[jj][:], rz[:, jj:jj + 1]
                    )
                    pending_probs.append((p_sb, h, jt))
                    pst = ps_t.tile([PDIM, T2], f32, tag="pt")
                    for tc_ in range(NTC):
                        nc.tensor.transpose(
                            pst[:, tc_ * PDIM:(tc_ + 1) * PDIM].bitcast(f32r),
                            p_sb[:, tc_ * PDIM:(tc_ + 1) * PDIM],
                            identr[:],
                        )
                    nc.vector.tensor_copy(
                        ptT[:, :, jj * PDIM:(jj + 1) * PDIM],
                        pst[:].rearrange("p (tc t) -> p tc t", t=PDIM),
                    )
                # y^T_h[span] = v_h^T @ P^T
                psy = ps_y.tile([Dh, 512], f32, tag="py")
                for tc_ in range(NTC):
                    nc.tensor.matmul(
                        psy[:],
                        v_sb[:, tc_, h * Dh:(h + 1) * Dh],
                        ptT[:, tc_, :],
                        start=(tc_ == 0),
                        stop=(tc_ == NTC - 1),
                    )
                pending_yT[0] = (psy, h, sp)

            if sp + 1 < NSPAN:
                qTs = qTs_next
                Ls_cur = Ls_next
        emit_pending_prob(len(pending_probs))
        flush_yT()
        emit_outproj(NSPAN - 1)

    nc.compile()
    bacc.get_activation_tables = _orig_tabs
    return nc


_NC_CACHE = {}


def kernel(query, query_mask, memory, memory_mask, attn_prior, Wq, Wkv, Wo):
    from concourse.bass_utils import run_bass_kernel_spmd

    query = np.asarray(query, dtype=np.float32)
    memory = np.asarray(memory, dtype=np.float32)
    attn_prior = np.asarray(attn_prior, dtype=np.float32)
    scale = np.float32(Dh ** -0.5)
    wq_s = np.ascontiguousarray(np.asarray(Wq, dtype=np.float32) * scale)
    wkv_h = np.ascontiguousarray(np.asarray(Wkv, dtype=np.float32))
    wo_h = np.ascontiguousarray(np.asarray(Wo, dtype=np.float32))

    if "nc" not in _NC_CACHE:
        _NC_CACHE["nc"] = _build_nc()
    nc = _NC_CACHE["nc"]

    in_maps = [
        {
            "query": np.ascontiguousarray(query[b]),
            "memory": np.ascontiguousarray(memory[b]),
            "prior": np.ascontiguousarray(attn_prior[b]),
            "wq": wq_s,
            "wkv": wkv_h,
            "wo": wo_h,
        }
        for b in range(B)
    ]
    res = run_bass_kernel_spmd(nc, in_maps, core_ids=list(range(B)))
    _NC_CACHE["last_result"] = res
    out = np.stack([r["out"] for r in res.results])
    prob = np.stack([r["prob"] for r in res.results])
    score = np.stack([r["score"] for r in res.results])
    return out, prob, score


# revision 74
# speedup vs baseline: 1.2365x; 1.0055x over previous
"""Trainium2 Bass kernel for prior-fused cross-attention.

Math (per batch b, head h):
  q  = query @ (Wq * Dh^-0.5)            # scale folded into Wq on host
  k,v = split(memory @ Wkv)
  s  = q_h @ k_h^T                       # == attn_score output (mask all-ones)
  L  = ln(prior + eps)
  P  = softmax(s + L)                    # == softmax(log_softmax(s) + L)
  y  = P_h @ v_h ;  out = y @ Wo

Layout strategy (per core = one batch element):
  - matmuls in float32r (1 cyc/row at N>=512); every matmul operand tile is
    declared float32r so its producer (copy/activation/DMA) does the rounding
    the BIR verifier demands.
  - fully pipelined over 4 t1-spans of 512: per span, transpose query on PE,
    project qT, ln(prior), then per head: s into one PSUM bank (copied out for
    the attn_score output) and s+L into another (identity-weight matmul
    accumulate) so exp never waits on the score copy/DMA; exp on ACT with
    accum_out giving the softmax denominator Z for free; P normalized with a
    per-partition tensor_scalar (split gpsimd/DVE); P transposed on PE to feed
    the P@v matmul; y kept transposed [c-part, t1-free] so the Wo matmul
    consumes it as lhsT; out-projection folded into each span.
"""

import numpy as np

B, T1, T2, C, H = 8, 2048, 512, 512, 8
Dh = C // H
EPS = 1e-8
PDIM = 128
NT1 = T1 // PDIM      # 16 t1 tiles
NSPAN = T1 // 512     # 4 t1 spans of 512
NCC = C // PDIM       # 4 contraction chunks
NTC = T2 // PDIM      # 4 t2 chunks
JJ = 512 // PDIM      # 4 t1-tiles per span


def _build_nc():
    from contextlib import ExitStack

    import concourse.bacc as bacc
    import concourse.mybir as mybir
    import concourse.tile as tile
    from concourse.masks import make_identity

    f32 = mybir.dt.float32
    f32r = mybir.dt.float32r
    AF = mybir.ActivationFunctionType

    # Prefer the act-function set holding Ln+Exp+Copy together so per-span
    # interleaving of ln/exp/copy needs a single table load.
    import concourse.hw_specs as _hw
    _orig_tabs = _hw.get_activation_tables

    def _tabs(arch, *a, **k):
        # Keep dict order (act_func_set_id = insertion index!) but strip
        # Ln/Exp from every other set so the selector lands on the combined
        # Ln+Exp+Copy set, minimizing runtime table reloads.
        import concourse.mybir as _mb
        tabs = dict(_orig_tabs(arch, *a, **k))
        strip = {_mb.ActivationFunctionType.Ln, _mb.ActivationFunctionType.Exp}
        return {
            name: (fns if name == 'natural_log_exp_and_others'
                   else set(fns) - strip)
            for name, fns in tabs.items()
        }

    bacc.get_activation_tables = _tabs

    nc = bacc.Bacc()
    query = nc.dram_tensor("query", [T1, C], f32, kind="ExternalInput")
    memory = nc.dram_tensor("memory", [T2, C], f32, kind="ExternalInput")
    prior = nc.dram_tensor("prior", [T1, T2], f32, kind="ExternalInput")
    wq = nc.dram_tensor("wq", [C, C], f32r, kind="ExternalInput")  # pre-scaled
    wkv = nc.dram_tensor("wkv", [C, 2 * C], f32r, kind="ExternalInput")
    wo = nc.dram_tensor("wo", [C, C], f32r, kind="ExternalInput")
    out = nc.dram_tensor("out", [T1, C], f32, kind="ExternalOutput")
    prob = nc.dram_tensor("prob", [H, T1, T2], f32, kind="ExternalOutput")
    score = nc.dram_tensor("score", [H, T1, T2], f32, kind="ExternalOutput")

    with tile.TileContext(nc) as tc, ExitStack() as ctx:
        singles = ctx.enter_context(tc.tile_pool(name="singles", bufs=1))
        ps_s = ctx.enter_context(tc.tile_pool(name="ps_s", bufs=3, space="PSUM"))
        ps_e = ctx.enter_context(tc.tile_pool(name="ps_e", bufs=2, space="PSUM"))
        ps_t = ctx.enter_context(tc.tile_pool(name="ps_t", bufs=2, space="PSUM"))
        ps_y = ctx.enter_context(tc.tile_pool(name="ps_y", bufs=1, space="PSUM"))

        ident = singles.tile([PDIM, PDIM], f32)
        make_identity(nc, ident[:])
        identr = singles.tile([PDIM, PDIM], f32r)
        nc.vector.tensor_copy(identr[:], ident[:])
        eps_b = singles.tile([PDIM, 1], f32)
        nc.vector.memset(eps_b[:], EPS)

        kT = singles.tile([PDIM, NCC, T2], f32r)
        v_sb = singles.tile([PDIM, NTC, C], f32r)
        wq_sb = singles.tile([PDIM, NCC, C], f32r)
        wo_sb = singles.tile([PDIM, NCC, C], f32r)

        # main pools first so the kv-header scope can close mid-stream (LIFO)
        poolIn = ctx.enter_context(tc.tile_pool(name="poolIn", bufs=3))
        poolQT = ctx.enter_context(tc.tile_pool(name="poolQT", bufs=2))
        poolL = ctx.enter_context(tc.tile_pool(name="poolL", bufs=2))
        poolB = ctx.enter_context(tc.tile_pool(name="poolB", bufs=1))
        yT = poolB.tile([PDIM, NCC, T1], f32r, tag="yT")
        poolE = ctx.enter_context(tc.tile_pool(name="poolE", bufs=8))
        poolS = ctx.enter_context(tc.tile_pool(name="poolS", bufs=3))
        poolP = ctx.enter_context(tc.tile_pool(name="poolP", bufs=6))
        poolPT = ctx.enter_context(tc.tile_pool(name="poolPT", bufs=2))
        poolO = ctx.enter_context(tc.tile_pool(name="poolO", bufs=2))
        poolZ = ctx.enter_context(tc.tile_pool(name="poolZ", bufs=4))

        # ----- kv header scope: memT + split wkv, closed once kT/v are done --
        ctxKV = ctx.enter_context(ExitStack())
        poolKV = ctxKV.enter_context(tc.tile_pool(name="poolKV", bufs=1))
        ldM = ctxKV.enter_context(tc.tile_pool(name="ldM", bufs=2))
        memT = poolKV.tile([PDIM, NCC, T2], f32r, tag="memT")
        for mt in range(NTC):
            mtl = ldM.tile([PDIM, C], f32r, tag="ldM")
            nc.sync.dma_start(mtl[:], memory[mt * PDIM:(mt + 1) * PDIM, :].bitcast(f32r))
            pt = ps_t.tile([PDIM, C], f32, tag="pt")
            for cc in range(NCC):
                nc.tensor.transpose(
                    pt[:, cc * PDIM:(cc + 1) * PDIM].bitcast(f32r),
                    mtl[:, cc * PDIM:(cc + 1) * PDIM],
                    identr[:],
                )
            nc.vector.tensor_copy(
                memT[:, :, mt * PDIM:(mt + 1) * PDIM],
                pt[:].rearrange("p (cc t) -> p cc t", t=PDIM),
            )
        wk_sb = poolKV.tile([PDIM, NCC, C], f32r, tag="wk")
        nc.sync.dma_start(
            wk_sb[:], wkv[:, 0:C].rearrange("(cc p) j -> p cc j", p=PDIM)
        )
        wv_sb = poolKV.tile([PDIM, NCC, C], f32r, tag="wv")

        def emit_kT(hps):
            for hp in hps:
                ps = ps_s.tile([PDIM, T2], f32, tag="ps")
                for cc in range(NCC):
                    nc.tensor.matmul(
                        ps[:],
                        wk_sb[:, cc, hp * PDIM:(hp + 1) * PDIM],
                        memT[:, cc, :],
                        start=(cc == 0),
                        stop=(cc == NCC - 1),
                    )
                nc.vector.tensor_copy(kT[:, hp, :], ps[:])

        def emit_v():
            for tc_ in range(NTC):
                ps = ps_s.tile([PDIM, C], f32, tag="ps")
                for cc in range(NCC):
                    nc.tensor.matmul(
                        ps[:],
                        memT[:, cc, tc_ * PDIM:(tc_ + 1) * PDIM],
                        wv_sb[:, cc, :],
                        start=(cc == 0),
                        stop=(cc == NCC - 1),
                    )
                nc.vector.tensor_copy(v_sb[:, tc_, :], ps[:])

        emit_kT([0])
        nc.sync.dma_start(wq_sb[:], wq[:].rearrange("(cc p) j -> p cc j", p=PDIM))

        def emit_queryT(sp):
            queryT = poolQT.tile([PDIM, NCC, 512], f32r, tag="queryT")
            for jj in range(JJ):
                jt = sp * JJ + jj
                qt = poolIn.tile([PDIM, C], f32r, tag="ldA")
                nc.sync.dma_start(qt[:], query[jt * PDIM:(jt + 1) * PDIM, :].bitcast(f32r))
                pt = ps_t.tile([PDIM, C], f32, tag="pt")
                for cc in range(NCC):
                    nc.tensor.transpose(
                        pt[:, cc * PDIM:(cc + 1) * PDIM].bitcast(f32r),
                        qt[:, cc * PDIM:(cc + 1) * PDIM],
                        identr[:],
                    )
                nc.vector.tensor_copy(
                    queryT[:, :, jj * PDIM:(jj + 1) * PDIM],
                    pt[:].rearrange("p (cc t) -> p cc t", t=PDIM),
                )
            return queryT

        def emit_qT(queryT, qTs=None, hps=range(NCC)):
            if qTs is None:
                qTs = poolQT.tile([PDIM, NCC, 512], f32r, tag="qT")
            for hp in hps:
                ps = ps_s.tile([PDIM, 512], f32, tag="ps")
                for cc in range(NCC):
                    nc.tensor.matmul(
                        ps[:],
                        wq_sb[:, cc, hp * PDIM:(hp + 1) * PDIM],
                        queryT[:, cc, :],
                        start=(cc == 0),
                        stop=(cc == NCC - 1),
                    )
                nc.scalar.copy(qTs[:, hp, :], ps[:])
            return qTs

        def emit_outproj(sp):
            for jj in range(JJ):
                jt = sp * JJ + jj
                ps = ps_s.tile([PDIM, C], f32, tag="ps")
                for cc in range(NCC):
                    nc.tensor.matmul(
                        ps[:],
                        yT[:, cc, jt * PDIM:(jt + 1) * PDIM],
                        wo_sb[:, cc, :],
                        start=(cc == 0),
                        stop=(cc == NCC - 1),
                    )
                o_sb = poolO.tile([PDIM, C], f32)
                nc.scalar.copy(o_sb[:], ps[:])
                nc.sync.dma_start(out[jt * PDIM:(jt + 1) * PDIM, :], o_sb[:])

        def emit_L(sp):
            Ls = poolL.tile([PDIM, JJ, T2], f32r, tag="L")
            for jj in range(JJ):
                jt = sp * JJ + jj
                ptile = poolIn.tile([PDIM, T2], f32, tag="ldP")
                nc.sync.dma_start(ptile[:], prior[jt * PDIM:(jt + 1) * PDIM, :])
                nc.scalar.activation(
                    Ls[:, jj, :], ptile[:], AF.Ln, bias=eps_b[:]
                )
            return Ls

        pending_probs = []

        def emit_pending_prob(n):
            for _ in range(min(n, len(pending_probs))):
                p_sb_, h_, jt_ = pending_probs.pop(0)
                nc.sync.dma_start(
                    prob[h_, jt_ * PDIM:(jt_ + 1) * PDIM, :].bitcast(f32r),
                    p_sb_[:],
                )

        pending_yT = [None]

        def flush_yT():
            if pending_yT[0] is not None:
                psy_, h_, sp_ = pending_yT[0]
                nc.vector.tensor_copy(
                    yT[(h_ % 2) * Dh:(h_ % 2 + 1) * Dh, h_ // 2,
                       sp_ * 512:(sp_ + 1) * 512],
                    psy_[:],
                )
                pending_yT[0] = None

        queryT_next = None
        qTs_next = None
        Ls_next = None
        queryT0 = emit_queryT(0)
        qTs = emit_qT(queryT0, hps=[0])
        Ls_cur = emit_L(0)
        nc.sync.dma_start(
            wv_sb[:], wkv[:, C:2 * C].rearrange("(cc p) j -> p cc j", p=PDIM)
        )
        nc.sync.dma_start(wo_sb[:], wo[:].rearrange("(cc p) j -> p cc j", p=PDIM))
        for sp in range(NSPAN):
            for h in range(H):
                if sp == 0 and h in (1, 3, 5):
                    hp = (h + 1) // 2
                    emit_kT([hp])
                    emit_qT(queryT0, qTs=qTs, hps=[hp])
                    if h == 5:
                        ctxKV.close()
                if h == 1 and sp + 1 < NSPAN:
                    Ls_next = emit_L(sp + 1)
                if h == 2 and sp + 1 < NSPAN:
                    queryT_next = emit_queryT(sp + 1)
                if h == 4 and sp + 1 < NSPAN:
                    qTs_next = emit_qT(queryT_next)
                flush_yT()
                if h == 1 and sp >= 1:
                    emit_outproj(sp - 1)
                hq = qTs[(h % 2) * Dh:(h % 2 + 1) * Dh, h // 2, :]
                hk = kT[(h % 2) * Dh:(h % 2 + 1) * Dh, h // 2, :]
                e_tiles = []
                zt = poolZ.tile([PDIM, JJ], f32, tag="z")
                rz = poolZ.tile([PDIM, JJ], f32, tag="rz")
                for jj in range(JJ):
                    jt = sp * JJ + jj
                    # score path
                    pss = ps_s.tile([PDIM, T2], f32, tag="ps")
                    nc.tensor.matmul(
                        pss[:],
                        hq[:, jj * PDIM:(jj + 1) * PDIM],
                        hk,
                        start=True,
                        stop=True,
                    )
                    s_sb = poolS.tile([PDIM, T2], f32)
                    if jj != 0:
                        nc.scalar.copy(s_sb[:], pss[:])
                    else:
                        nc.vector.tensor_copy(s_sb[:], pss[:])
                    nc.sync.dma_start(
                        score[h, jt * PDIM:(jt + 1) * PDIM, :], s_sb[:]
                    )
                    emit_pending_prob(1)
                    # exp path: recompute s, add L, exp (+Z via accum_out)
                    pse = ps_e.tile([PDIM, T2], f32, tag="pe")
                    nc.tensor.matmul(
                        pse[:],
                        hq[:, jj * PDIM:(jj + 1) * PDIM],
                        hk,
                        start=True,
                        stop=False,
                    )
                    nc.tensor.matmul(
                        pse[:],
                        identr[:],
                        Ls_cur[:, jj, :],
                        start=False,
                        stop=True,
                    )
                    e_sb = poolE.tile([PDIM, T2], f32)
                    nc.scalar.activation(
                        e_sb[:], pse[:], AF.Exp,
                        accum_out=zt[:, jj:jj + 1],
                    )
                    # per-tile reciprocal so the normalize of tile jj only
                    # waits on its own exp, not all four
                    nc.vector.reciprocal(rz[:, jj:jj + 1], zt[:, jj:jj + 1])
                    e_tiles.append(e_sb)
                if sp == 0 and h == 0:
                    emit_v()
                ptT = poolPT.tile([PDIM, NTC, 512], f32r)
                for jj in range(JJ):
                    jt = sp * JJ + jj
                    p_sb = poolP.tile([PDIM, T2], f32r)
                    nc.vector.tensor_scalar_mul(
                        p_sb[:], e_tiles[jj][:], rz[:, jj:jj + 1]
                    )
                    pending_probs.append((p_sb, h, jt))
                    pst = ps_t.tile([PDIM, T2], f32, tag="pt")
                    for tc_ in range(NTC):
                        nc.tensor.transpose(
                            pst[:, tc_ * PDIM:(tc_ + 1) * PDIM].bitcast(f32r),
                            p_sb[:, tc_ * PDIM:(tc_ + 1) * PDIM],
                            identr[:],
                        )
                    nc.vector.tensor_copy(
                        ptT[:, :, jj * PDIM:(jj + 1) * PDIM],
                        pst[:].rearrange("p (tc t) -> p tc t", t=PDIM),
                    )
                # y^T_h[span] = v_h^T @ P^T
                psy = ps_y.tile([Dh, 512], f32, tag="py")
                for tc_ in range(NTC):
                    nc.tensor.matmul(
                        psy[:],
                        v_sb[:, tc_, h * Dh:(h + 1) * Dh],
                        ptT[:, tc_, :],
                        start=(tc_ == 0),
                        stop=(tc_ == NTC - 1),
                    )
                pending_yT[0] = (psy, h, sp)

            if sp + 1 < NSPAN:
                qTs = qTs_next
                Ls_cur = Ls_next
        emit_pending_prob(len(pending_probs))
        flush_yT()
        emit_outproj(NSPAN - 1)

    nc.compile()
    bacc.get_activation_tables = _orig_tabs
    return nc


_NC_CACHE = {}


def kernel(query, query_mask, memory, memory_mask, attn_prior, Wq, Wkv, Wo):
    from concourse.bass_utils import run_bass_kernel_spmd

    query = np.asarray(query, dtype=np.float32)
    memory = np.asarray(memory, dtype=np.float32)
    attn_prior = np.asarray(attn_prior, dtype=np.float32)
    scale = np.float32(Dh ** -0.5)
    wq_s = np.ascontiguousarray(np.asarray(Wq, dtype=np.float32) * scale)
    wkv_h = np.ascontiguousarray(np.asarray(Wkv, dtype=np.float32))
    wo_h = np.ascontiguousarray(np.asarray(Wo, dtype=np.float32))

    if "nc" not in _NC_CACHE:
        _NC_CACHE["nc"] = _build_nc()
    nc = _NC_CACHE["nc"]

    in_maps = [
        {
            "query": np.ascontiguousarray(query[b]),
            "memory": np.ascontiguousarray(memory[b]),
            "prior": np.ascontiguousarray(attn_prior[b]),
            "wq": wq_s,
            "wkv": wkv_h,
            "wo": wo_h,
        }
        for b in range(B)
    ]
    res = run_bass_kernel_spmd(nc, in_maps, core_ids=list(range(B)))
    _NC_CACHE["last_result"] = res
    out = np.stack([r["out"] for r in res.results])
    prob = np.stack([r["prob"] for r in res.results])
    score = np.stack([r["score"] for r in res.results])
    return out, prob, score


# revision 85
# speedup vs baseline: 1.2587x; 1.0180x over previous
"""Trainium2 Bass kernel for prior-fused cross-attention.

Math (per batch b, head h):
  q  = query @ (Wq * Dh^-0.5)            # scale folded into Wq on host
  k,v = split(memory @ Wkv)
  s  = q_h @ k_h^T                       # == attn_score output (mask all-ones)
  L  = ln(prior + eps)
  P  = softmax(s + L)                    # == softmax(log_softmax(s) + L)
  y  = P_h @ v_h ;  out = y @ Wo

Layout strategy (per core = one batch element):
  - matmuls in float32r (1 cyc/row at N>=512); every matmul operand tile is
    declared float32r so its producer (copy/activation/DMA) does the rounding
    the BIR verifier demands.
  - fully pipelined over 4 t1-spans of 512: per span, transpose query on PE,
    project qT, ln(prior), then per head: s into one PSUM bank (copied out for
    the attn_score output) and s+L into another (identity-weight matmul
    accumulate) so exp never waits on the score copy/DMA; exp on ACT with
    accum_out giving the softmax denominator Z for free; P normalized with a
    per-partition tensor_scalar (split gpsimd/DVE); P transposed on PE to feed
    the P@v matmul; y kept transposed [c-part, t1-free] so the Wo matmul
    consumes it as lhsT; out-projection folded into each span.
"""

import numpy as np

B, T1, T2, C, H = 8, 2048, 512, 512, 8
Dh = C // H
EPS = 1e-8
PDIM = 128
NT1 = T1 // PDIM      # 16 t1 tiles
NSPAN = T1 // 512     # 4 t1 spans of 512
NCC = C // PDIM       # 4 contraction chunks
NTC = T2 // PDIM      # 4 t2 chunks
JJ = 512 // PDIM      # 4 t1-tiles per span


def _build_nc():
    from contextlib import ExitStack

    import concourse.bacc as bacc
    import concourse.mybir as mybir
    import concourse.tile as tile
    from concourse.masks import make_identity

    f32 = mybir.dt.float32
    f32r = mybir.dt.float32r
    AF = mybir.ActivationFunctionType

    # Prefer the act-function set holding Ln+Exp+Copy together so per-span
    # interleaving of ln/exp/copy needs a single table load.
    import concourse.hw_specs as _hw
    _orig_tabs = _hw.get_activation_tables

    def _tabs(arch, *a, **k):
        # Keep dict order (act_func_set_id = insertion index!) but strip
        # Ln/Exp from every other set so the selector lands on the combined
        # Ln+Exp+Copy set, minimizing runtime table reloads.
        import concourse.mybir as _mb
        tabs = dict(_orig_tabs(arch, *a, **k))
        strip = {_mb.ActivationFunctionType.Ln, _mb.ActivationFunctionType.Exp}
        return {
            name: (fns if name == 'natural_log_exp_and_others'
                   else set(fns) - strip)
            for name, fns in tabs.items()
        }

    bacc.get_activation_tables = _tabs

    nc = bacc.Bacc()
    query = nc.dram_tensor("query", [T1, C], f32, kind="ExternalInput")
    memory = nc.dram_tensor("memory", [T2, C], f32, kind="ExternalInput")
    prior = nc.dram_tensor("prior", [T1, T2], f32, kind="ExternalInput")
    wq = nc.dram_tensor("wq", [C, C], f32r, kind="ExternalInput")  # pre-scaled
    wkv = nc.dram_tensor("wkv", [C, 2 * C], f32r, kind="ExternalInput")
    wo = nc.dram_tensor("wo", [C, C], f32r, kind="ExternalInput")
    out = nc.dram_tensor("out", [T1, C], f32, kind="ExternalOutput")
    prob = nc.dram_tensor("prob", [H, T1, T2], f32, kind="ExternalOutput")
    score = nc.dram_tensor("score", [H, T1, T2], f32, kind="ExternalOutput")

    with tile.TileContext(nc) as tc, ExitStack() as ctx:
        singles = ctx.enter_context(tc.tile_pool(name="singles", bufs=1))
        ps_s = ctx.enter_context(tc.tile_pool(name="ps_s", bufs=3, space="PSUM"))
        ps_e = ctx.enter_context(tc.tile_pool(name="ps_e", bufs=2, space="PSUM"))
        ps_t = ctx.enter_context(tc.tile_pool(name="ps_t", bufs=2, space="PSUM"))
        ps_y = ctx.enter_context(tc.tile_pool(name="ps_y", bufs=1, space="PSUM"))

        ident = singles.tile([PDIM, PDIM], f32)
        make_identity(nc, ident[:])
        identr = singles.tile([PDIM, PDIM], f32r)
        nc.vector.tensor_copy(identr[:], ident[:])
        eps_b = singles.tile([PDIM, 1], f32)
        nc.vector.memset(eps_b[:], EPS)

        kT = singles.tile([PDIM, NCC, T2], f32r)
        v_sb = singles.tile([PDIM, NTC, C], f32r)
        wq_sb = singles.tile([PDIM, NCC, C], f32r)
        wo_sb = singles.tile([PDIM, NCC, C], f32r)

        # main pools first so the kv-header scope can close mid-stream (LIFO)
        poolIn = ctx.enter_context(tc.tile_pool(name="poolIn", bufs=3))
        poolQT = ctx.enter_context(tc.tile_pool(name="poolQT", bufs=2))
        poolL = ctx.enter_context(tc.tile_pool(name="poolL", bufs=2))
        poolB = ctx.enter_context(tc.tile_pool(name="poolB", bufs=1))
        yT = poolB.tile([PDIM, NCC, T1], f32r, tag="yT")
        poolE = ctx.enter_context(tc.tile_pool(name="poolE", bufs=8))
        poolS = ctx.enter_context(tc.tile_pool(name="poolS", bufs=3))
        poolP = ctx.enter_context(tc.tile_pool(name="poolP", bufs=6))
        poolPT = ctx.enter_context(tc.tile_pool(name="poolPT", bufs=2))
        poolO = ctx.enter_context(tc.tile_pool(name="poolO", bufs=2))
        poolZ = ctx.enter_context(tc.tile_pool(name="poolZ", bufs=4))

        # ----- kv header scope: memT + split wkv, closed once kT/v are done --
        ctxKV = ctx.enter_context(ExitStack())
        poolKV = ctxKV.enter_context(tc.tile_pool(name="poolKV", bufs=1))
        ldM = ctxKV.enter_context(tc.tile_pool(name="ldM", bufs=2))
        memT = poolKV.tile([PDIM, NCC, T2], f32r, tag="memT")
        for mt in range(NTC):
            mtl = ldM.tile([PDIM, C], f32r, tag="ldM")
            nc.sync.dma_start(mtl[:], memory[mt * PDIM:(mt + 1) * PDIM, :].bitcast(f32r))
            pt = ps_t.tile([PDIM, C], f32, tag="pt")
            for cc in range(NCC):
                nc.tensor.transpose(
                    pt[:, cc * PDIM:(cc + 1) * PDIM].bitcast(f32r),
                    mtl[:, cc * PDIM:(cc + 1) * PDIM],
                    identr[:],
                )
            nc.vector.tensor_copy(
                memT[:, :, mt * PDIM:(mt + 1) * PDIM],
                pt[:].rearrange("p (cc t) -> p cc t", t=PDIM),
            )
        wk_sb = poolKV.tile([PDIM, NCC, C], f32r, tag="wk")
        nc.sync.dma_start(
            wk_sb[:], wkv[:, 0:C].rearrange("(cc p) j -> p cc j", p=PDIM)
        )
        wv_sb = poolKV.tile([PDIM, NCC, C], f32r, tag="wv")

        def emit_kT(hps):
            for hp in hps:
                ps = ps_s.tile([PDIM, T2], f32, tag="ps")
                for cc in range(NCC):
                    nc.tensor.matmul(
                        ps[:],
                        wk_sb[:, cc, hp * PDIM:(hp + 1) * PDIM],
                        memT[:, cc, :],
                        start=(cc == 0),
                        stop=(cc == NCC - 1),
                    )
                nc.vector.tensor_copy(kT[:, hp, :], ps[:])

        def emit_v():
            for tc_ in range(NTC):
                ps = ps_s.tile([PDIM, C], f32, tag="ps")
                for cc in range(NCC):
                    nc.tensor.matmul(
                        ps[:],
                        memT[:, cc, tc_ * PDIM:(tc_ + 1) * PDIM],
                        wv_sb[:, cc, :],
                        start=(cc == 0),
                        stop=(cc == NCC - 1),
                    )
                nc.vector.tensor_copy(v_sb[:, tc_, :], ps[:])

        emit_kT([0])
        nc.sync.dma_start(wq_sb[:], wq[:].rearrange("(cc p) j -> p cc j", p=PDIM))

        def emit_queryT(sp):
            queryT = poolQT.tile([PDIM, NCC, 512], f32r, tag="queryT")
            for jj in range(JJ):
                jt = sp * JJ + jj
                qt = poolIn.tile([PDIM, C], f32r, tag="ldA")
                nc.sync.dma_start(qt[:], query[jt * PDIM:(jt + 1) * PDIM, :].bitcast(f32r))
                pt = ps_t.tile([PDIM, C], f32, tag="pt")
                for cc in range(NCC):
                    nc.tensor.transpose(
                        pt[:, cc * PDIM:(cc + 1) * PDIM].bitcast(f32r),
                        qt[:, cc * PDIM:(cc + 1) * PDIM],
                        identr[:],
                    )
                nc.vector.tensor_copy(
                    queryT[:, :, jj * PDIM:(jj + 1) * PDIM],
                    pt[:].rearrange("p (cc t) -> p cc t", t=PDIM),
                )
            return queryT

        def emit_qT(queryT, qTs=None, hps=range(NCC)):
            if qTs is None:
                qTs = poolQT.tile([PDIM, NCC, 512], f32r, tag="qT")
            for hp in hps:
                ps = ps_s.tile([PDIM, 512], f32, tag="ps")
                for cc in range(NCC):
                    nc.tensor.matmul(
                        ps[:],
                        wq_sb[:, cc, hp * PDIM:(hp + 1) * PDIM],
                        queryT[:, cc, :],
                        start=(cc == 0),
                        stop=(cc == NCC - 1),
                    )
                nc.scalar.copy(qTs[:, hp, :], ps[:])
            return qTs

        def emit_outproj(sp):
            for jj in range(JJ):
                jt = sp * JJ + jj
                ps = ps_s.tile([PDIM, C], f32, tag="ps")
                for cc in range(NCC):
                    nc.tensor.matmul(
                        ps[:],
                        yT[:, cc, jt * PDIM:(jt + 1) * PDIM],
                        wo_sb[:, cc, :],
                        start=(cc == 0),
                        stop=(cc == NCC - 1),
                    )
                o_sb = poolO.tile([PDIM, C], f32)
                nc.scalar.copy(o_sb[:], ps[:])
                nc.sync.dma_start(out[jt * PDIM:(jt + 1) * PDIM, :], o_sb[:])

        def emit_L(sp):
            Ls = poolL.tile([PDIM, JJ, T2], f32r, tag="L")
            for jj in range(JJ):
                jt = sp * JJ + jj
                ptile = poolIn.tile([PDIM, T2], f32, tag="ldP")
                nc.sync.dma_start(ptile[:], prior[jt * PDIM:(jt + 1) * PDIM, :])
                nc.scalar.activation(
                    Ls[:, jj, :], ptile[:], AF.Ln, bias=eps_b[:]
                )
            return Ls

        pending_probs = []

        def emit_pending_prob(n):
            for _ in range(min(n, len(pending_probs))):
                p_sb_, h_, jt_ = pending_probs.pop(0)
                nc.sync.dma_start(
                    prob[h_, jt_ * PDIM:(jt_ + 1) * PDIM, :].bitcast(f32r),
                    p_sb_[:],
                )

        pending_yT = [None]

        def flush_yT():
            if pending_yT[0] is not None:
                psy_, h_, sp_ = pending_yT[0]
                nc.vector.tensor_copy(
                    yT[(h_ % 2) * Dh:(h_ % 2 + 1) * Dh, h_ // 2,
                       sp_ * 512:(sp_ + 1) * 512],
                    psy_[:],
                )
                pending_yT[0] = None

        queryT_next = None
        qTs_next = None
        Ls_next = None
        queryT0 = emit_queryT(0)
        qTs = emit_qT(queryT0, hps=[0])
        Ls_cur = emit_L(0)
        nc.sync.dma_start(
            wv_sb[:], wkv[:, C:2 * C].rearrange("(cc p) j -> p cc j", p=PDIM)
        )
        nc.sync.dma_start(wo_sb[:], wo[:].rearrange("(cc p) j -> p cc j", p=PDIM))
        for sp in range(NSPAN):
            for h in range(H):
                if sp == 0 and h in (1, 3, 5):
                    hp = (h + 1) // 2
                    emit_kT([hp])
                    emit_qT(queryT0, qTs=qTs, hps=[hp])
                    if h == 5:
                        ctxKV.close()
                if h == 1 and sp + 1 < NSPAN:
                    Ls_next = emit_L(sp + 1)
                if h == 2 and sp + 1 < NSPAN:
                    queryT_next = emit_queryT(sp + 1)
                if h == 4 and sp + 1 < NSPAN:
                    qTs_next = emit_qT(queryT_next)
                flush_yT()
                if h == 1 and sp >= 1:
                    emit_outproj(sp - 1)
                hq = qTs[(h % 2) * Dh:(h % 2 + 1) * Dh, h // 2, :]
                hk = kT[(h % 2) * Dh:(h % 2 + 1) * Dh, h // 2, :]
                e_tiles = []
                zt = poolZ.tile([PDIM, JJ], f32, tag="z")
                rz = poolZ.tile([PDIM, JJ], f32, tag="rz")
                for jj in range(JJ):
                    jt = sp * JJ + jj
                    # score path
                    pss = ps_s.tile([PDIM, T2], f32, tag="ps")
                    nc.tensor.matmul(
                        pss[:],
                        hq[:, jj * PDIM:(jj + 1) * PDIM],
                        hk,
                        start=True,
                        stop=True,
                    )
                    s_sb = poolS.tile([PDIM, T2], f32)
                    nc.scalar.copy(s_sb[:], pss[:])
                    nc.sync.dma_start(
                        score[h, jt * PDIM:(jt + 1) * PDIM, :], s_sb[:]
                    )
                    emit_pending_prob(1)
                    # exp path: recompute s, add L, exp (+Z via accum_out)
                    pse = ps_e.tile([PDIM, T2], f32, tag="pe")
                    nc.tensor.matmul(
                        pse[:],
                        hq[:, jj * PDIM:(jj + 1) * PDIM],
                        hk,
                        start=True,
                        stop=False,
                    )
                    nc.tensor.matmul(
                        pse[:],
                        identr[:],
                        Ls_cur[:, jj, :],
                        start=False,
                        stop=True,
                    )
                    e_sb = poolE.tile([PDIM, T2], f32)
                    nc.scalar.activation(
                        e_sb[:], pse[:], AF.Exp,
                        accum_out=zt[:, jj:jj + 1],
                    )
                    # per-tile reciprocal so the normalize of tile jj only
                    # waits on its own exp, not all four
                    nc.vector.reciprocal(rz[:, jj:jj + 1], zt[:, jj:jj + 1])
                    e_tiles.append(e_sb)
                if sp == 0 and h == 0:
                    emit_v()
                ptT = poolPT.tile([PDIM, NTC, 512], f32r)
                for jj in range(JJ):
                    jt = sp * JJ + jj
                    p_sb = poolP.tile([PDIM, T2], f32r)
                    nc.vector.tensor_scalar_mul(
                        p_sb[:], e_tiles[jj][:], rz[:, jj:jj + 1]
                    )
                    pending_probs.append((p_sb, h, jt))
                    pst = ps_t.tile([PDIM, T2], f32, tag="pt")
                    for tc_ in range(NTC):
                        nc.tensor.transpose(
                            pst[:, tc_ * PDIM:(tc_ + 1) * PDIM].bitcast(f32r),
                            p_sb[:, tc_ * PDIM:(tc_ + 1) * PDIM],
                            identr[:],
                        )
                    nc.vector.tensor_copy(
                        ptT[:, :, jj * PDIM:(jj + 1) * PDIM],
                        pst[:].rearrange("p (tc t) -> p tc t", t=PDIM),
                    )
                # y^T_h[span] = v_h^T @ P^T
                psy = ps_y.tile([Dh, 512], f32, tag="py")
                for tc_ in range(NTC):
                    nc.tensor.matmul(
                        psy[:],
                        v_sb[:, tc_, h * Dh:(h + 1) * Dh],
                        ptT[:, tc_, :],
                        start=(tc_ == 0),
                        stop=(tc_ == NTC - 1),
                    )
                pending_yT[0] = (psy, h, sp)

            if sp + 1 < NSPAN:
                qTs = qTs_next
                Ls_cur = Ls_next
        emit_pending_prob(len(pending_probs))
        flush_yT()
        emit_outproj(NSPAN - 1)

    nc.compile()
    bacc.get_activation_tables = _orig_tabs
    return nc


_NC_CACHE = {}


def kernel(query, query_mask, memory, memory_mask, attn_prior, Wq, Wkv, Wo):
    from concourse.bass_utils import run_bass_kernel_spmd

    query = np.asarray(query, dtype=np.float32)
    memory = np.asarray(memory, dtype=np.float32)
    attn_prior = np.asarray(attn_prior, dtype=np.float32)
    scale = np.float32(Dh ** -0.5)
    wq_s = np.ascontiguousarray(np.asarray(Wq, dtype=np.float32) * scale)
    wkv_h = np.ascontiguousarray(np.asarray(Wkv, dtype=np.float32))
    wo_h = np.ascontiguousarray(np.asarray(Wo, dtype=np.float32))

    if "nc" not in _NC_CACHE:
        _NC_CACHE["nc"] = _build_nc()
    nc = _NC_CACHE["nc"]

    in_maps = [
        {
            "query": np.ascontiguousarray(query[b]),
            "memory": np.ascontiguousarray(memory[b]),
            "prior": np.ascontiguousarray(attn_prior[b]),
            "wq": wq_s,
            "wkv": wkv_h,
            "wo": wo_h,
        }
        for b in range(B)
    ]
    res = run_bass_kernel_spmd(nc, in_maps, core_ids=list(range(B)))
    _NC_CACHE["last_result"] = res
    out = np.stack([r["out"] for r in res.results])
    prob = np.stack([r["prob"] for r in res.results])
    score = np.stack([r["score"] for r in res.results])
    return out, prob, score


# revision 95
# speedup vs baseline: 1.2680x; 1.0074x over previous
"""Trainium2 Bass kernel for prior-fused cross-attention.

Math (per batch b, head h):
  q  = query @ (Wq * Dh^-0.5)            # scale folded into Wq on host
  k,v = split(memory @ Wkv)
  s  = q_h @ k_h^T                       # == attn_score output (mask all-ones)
  L  = ln(prior + eps)
  P  = softmax(s + L)                    # == softmax(log_softmax(s) + L)
  y  = P_h @ v_h ;  out = y @ Wo

Layout strategy (per core = one batch element):
  - matmuls in float32r (1 cyc/row at N>=512); every matmul operand tile is
    declared float32r so its producer (copy/activation/DMA) does the rounding
    the BIR verifier demands.
  - fully pipelined over 4 t1-spans of 512: per span, transpose query on PE,
    project qT, ln(prior), then per head: s into one PSUM bank (copied out for
    the attn_score output) and s+L into another (identity-weight matmul
    accumulate) so exp never waits on the score copy/DMA; exp on ACT with
    accum_out giving the softmax denominator Z for free; P normalized with a
    per-partition tensor_scalar (split gpsimd/DVE); P transposed on PE to feed
    the P@v matmul; y kept transposed [c-part, t1-free] so the Wo matmul
    consumes it as lhsT; out-projection folded into each span.
"""

import numpy as np

B, T1, T2, C, H = 8, 2048, 512, 512, 8
Dh = C // H
EPS = 1e-8
PDIM = 128
NT1 = T1 // PDIM      # 16 t1 tiles
NSPAN = T1 // 512     # 4 t1 spans of 512
NCC = C // PDIM       # 4 contraction chunks
NTC = T2 // PDIM      # 4 t2 chunks
JJ = 512 // PDIM      # 4 t1-tiles per span


def _build_nc():
    from contextlib import ExitStack

    import concourse.bacc as bacc
    import concourse.mybir as mybir
    import concourse.tile as tile
    from concourse.masks import make_identity

    f32 = mybir.dt.float32
    f32r = mybir.dt.float32r
    AF = mybir.ActivationFunctionType

    # Prefer the act-function set holding Ln+Exp+Copy together so per-span
    # interleaving of ln/exp/copy needs a single table load.
    import concourse.hw_specs as _hw
    _orig_tabs = _hw.get_activation_tables

    def _tabs(arch, *a, **k):
        # Keep dict order (act_func_set_id = insertion index!) but strip
        # Ln/Exp from every other set so the selector lands on the combined
        # Ln+Exp+Copy set, minimizing runtime table reloads.
        import concourse.mybir as _mb
        tabs = dict(_orig_tabs(arch, *a, **k))
        strip = {_mb.ActivationFunctionType.Ln, _mb.ActivationFunctionType.Exp}
        return {
            name: (fns if name == 'natural_log_exp_and_others'
                   else set(fns) - strip)
            for name, fns in tabs.items()
        }

    bacc.get_activation_tables = _tabs

    nc = bacc.Bacc()
    query = nc.dram_tensor("query", [T1, C], f32, kind="ExternalInput")
    memory = nc.dram_tensor("memory", [T2, C], f32, kind="ExternalInput")
    prior = nc.dram_tensor("prior", [T1, T2], f32, kind="ExternalInput")
    wq = nc.dram_tensor("wq", [C, C], f32r, kind="ExternalInput")  # pre-scaled
    wkv = nc.dram_tensor("wkv", [C, 2 * C], f32r, kind="ExternalInput")
    wo = nc.dram_tensor("wo", [C, C], f32r, kind="ExternalInput")
    out = nc.dram_tensor("out", [T1, C], f32, kind="ExternalOutput")
    prob = nc.dram_tensor("prob", [H, T1, T2], f32, kind="ExternalOutput")
    score = nc.dram_tensor("score", [H, T1, T2], f32, kind="ExternalOutput")

    with tile.TileContext(nc) as tc, ExitStack() as ctx:
        singles = ctx.enter_context(tc.tile_pool(name="singles", bufs=1))
        ps_s = ctx.enter_context(tc.tile_pool(name="ps_s", bufs=3, space="PSUM"))
        ps_e = ctx.enter_context(tc.tile_pool(name="ps_e", bufs=2, space="PSUM"))
        ps_t = ctx.enter_context(tc.tile_pool(name="ps_t", bufs=2, space="PSUM"))
        ps_y = ctx.enter_context(tc.tile_pool(name="ps_y", bufs=1, space="PSUM"))

        ident = singles.tile([PDIM, PDIM], f32)
        make_identity(nc, ident[:])
        identr = singles.tile([PDIM, PDIM], f32r)
        nc.vector.tensor_copy(identr[:], ident[:])
        eps_b = singles.tile([PDIM, 1], f32)
        nc.vector.memset(eps_b[:], EPS)

        kT = singles.tile([PDIM, NCC, T2], f32r)
        v_sb = singles.tile([PDIM, NTC, C], f32r)
        wq_sb = singles.tile([PDIM, NCC, C], f32r)
        wo_sb = singles.tile([PDIM, NCC, C], f32r)

        # main pools first so the kv-header scope can close mid-stream (LIFO)
        poolIn = ctx.enter_context(tc.tile_pool(name="poolIn", bufs=3))
        poolQT = ctx.enter_context(tc.tile_pool(name="poolQT", bufs=2))
        poolL = ctx.enter_context(tc.tile_pool(name="poolL", bufs=2))
        poolB = ctx.enter_context(tc.tile_pool(name="poolB", bufs=1))
        yT = poolB.tile([PDIM, NCC, T1], f32r, tag="yT")
        poolE = ctx.enter_context(tc.tile_pool(name="poolE", bufs=7))
        poolS = ctx.enter_context(tc.tile_pool(name="poolS", bufs=3))
        poolP = ctx.enter_context(tc.tile_pool(name="poolP", bufs=6))
        poolPT = ctx.enter_context(tc.tile_pool(name="poolPT", bufs=2))
        poolO = ctx.enter_context(tc.tile_pool(name="poolO", bufs=2))
        poolZ = ctx.enter_context(tc.tile_pool(name="poolZ", bufs=4))

        # ----- kv header scope: memT + split wkv, closed once kT/v are done --
        ctxKV = ctx.enter_context(ExitStack())
        poolKV = ctxKV.enter_context(tc.tile_pool(name="poolKV", bufs=1))
        ldM = ctxKV.enter_context(tc.tile_pool(name="ldM", bufs=2))
        memT = poolKV.tile([PDIM, NCC, T2], f32r, tag="memT")
        for mt in range(NTC):
            mtl = ldM.tile([PDIM, C], f32r, tag="ldM")
            nc.sync.dma_start(mtl[:], memory[mt * PDIM:(mt + 1) * PDIM, :].bitcast(f32r))
            pt = ps_t.tile([PDIM, C], f32, tag="pt")
            for cc in range(NCC):
                nc.tensor.transpose(
                    pt[:, cc * PDIM:(cc + 1) * PDIM].bitcast(f32r),
                    mtl[:, cc * PDIM:(cc + 1) * PDIM],
                    identr[:],
                )
            nc.vector.tensor_copy(
                memT[:, :, mt * PDIM:(mt + 1) * PDIM],
                pt[:].rearrange("p (cc t) -> p cc t", t=PDIM),
            )
        wk_sb = poolKV.tile([PDIM, NCC, C], f32r, tag="wk")
        nc.sync.dma_start(
            wk_sb[:], wkv[:, 0:C].rearrange("(cc p) j -> p cc j", p=PDIM)
        )
        wv_sb = poolKV.tile([PDIM, NCC, C], f32r, tag="wv")

        def emit_kT(hps):
            for hp in hps:
                ps = ps_s.tile([PDIM, T2], f32, tag="ps")
                for cc in range(NCC):
                    nc.tensor.matmul(
                        ps[:],
                        wk_sb[:, cc, hp * PDIM:(hp + 1) * PDIM],
                        memT[:, cc, :],
                        start=(cc == 0),
                        stop=(cc == NCC - 1),
                    )
                nc.vector.tensor_copy(kT[:, hp, :], ps[:])

        def emit_v():
            for tc_ in range(NTC):
                ps = ps_s.tile([PDIM, C], f32, tag="ps")
                for cc in range(NCC):
                    nc.tensor.matmul(
                        ps[:],
                        memT[:, cc, tc_ * PDIM:(tc_ + 1) * PDIM],
                        wv_sb[:, cc, :],
                        start=(cc == 0),
                        stop=(cc == NCC - 1),
                    )
                nc.vector.tensor_copy(v_sb[:, tc_, :], ps[:])

        emit_kT([0])
        nc.sync.dma_start(wq_sb[:], wq[:].rearrange("(cc p) j -> p cc j", p=PDIM))

        def emit_queryT(sp):
            queryT = poolQT.tile([PDIM, NCC, 512], f32r, tag="queryT")
            for jj in range(JJ):
                jt = sp * JJ + jj
                qt = poolIn.tile([PDIM, C], f32r, tag="ldA")
                nc.sync.dma_start(qt[:], query[jt * PDIM:(jt + 1) * PDIM, :].bitcast(f32r))
                pt = ps_t.tile([PDIM, C], f32, tag="pt")
                for cc in range(NCC):
                    nc.tensor.transpose(
                        pt[:, cc * PDIM:(cc + 1) * PDIM].bitcast(f32r),
                        qt[:, cc * PDIM:(cc + 1) * PDIM],
                        identr[:],
                    )
                nc.vector.tensor_copy(
                    queryT[:, :, jj * PDIM:(jj + 1) * PDIM],
                    pt[:].rearrange("p (cc t) -> p cc t", t=PDIM),
                )
            return queryT

        def emit_qT(queryT, qTs=None, hps=range(NCC)):
            if qTs is None:
                qTs = poolQT.tile([PDIM, NCC, 512], f32r, tag="qT")
            for hp in hps:
                ps = ps_s.tile([PDIM, 512], f32, tag="ps")
                for cc in range(NCC):
                    nc.tensor.matmul(
                        ps[:],
                        wq_sb[:, cc, hp * PDIM:(hp + 1) * PDIM],
                        queryT[:, cc, :],
                        start=(cc == 0),
                        stop=(cc == NCC - 1),
                    )
                nc.scalar.copy(qTs[:, hp, :], ps[:])
            return qTs

        def emit_outproj(sp):
            for jj in range(JJ):
                jt = sp * JJ + jj
                ps = ps_s.tile([PDIM, C], f32, tag="ps")
                for cc in range(NCC):
                    nc.tensor.matmul(
                        ps[:],
                        yT[:, cc, jt * PDIM:(jt + 1) * PDIM],
                        wo_sb[:, cc, :],
                        start=(cc == 0),
                        stop=(cc == NCC - 1),
                    )
                o_sb = poolO.tile([PDIM, C], f32)
                nc.scalar.copy(o_sb[:], ps[:])
                nc.sync.dma_start(out[jt * PDIM:(jt + 1) * PDIM, :], o_sb[:])

        def emit_L(sp):
            Ls = poolL.tile([PDIM, JJ, T2], f32r, tag="L")
            for jj in range(JJ):
                jt = sp * JJ + jj
                ptile = poolIn.tile([PDIM, T2], f32, tag="ldP")
                nc.sync.dma_start(ptile[:], prior[jt * PDIM:(jt + 1) * PDIM, :])
                nc.scalar.activation(
                    Ls[:, jj, :], ptile[:], AF.Ln, bias=eps_b[:]
                )
            return Ls

        pending_probs = []

        def emit_pending_prob(n):
            for _ in range(min(n, len(pending_probs))):
                p_sb_, h_, jt_ = pending_probs.pop(0)
                nc.sync.dma_start(
                    prob[h_, jt_ * PDIM:(jt_ + 1) * PDIM, :].bitcast(f32r),
                    p_sb_[:],
                )

        pending_yT = [None]

        def flush_yT():
            if pending_yT[0] is not None:
                psy_, h_, sp_ = pending_yT[0]
                nc.vector.tensor_copy(
                    yT[(h_ % 2) * Dh:(h_ % 2 + 1) * Dh, h_ // 2,
                       sp_ * 512:(sp_ + 1) * 512],
                    psy_[:],
                )
                pending_yT[0] = None

        queryT_next = None
        qTs_next = None
        Ls_next = None
        queryT0 = emit_queryT(0)
        qTs = emit_qT(queryT0, hps=[0])
        Ls_cur = emit_L(0)
        nc.sync.dma_start(
            wv_sb[:], wkv[:, C:2 * C].rearrange("(cc p) j -> p cc j", p=PDIM)
        )
        nc.sync.dma_start(wo_sb[:], wo[:].rearrange("(cc p) j -> p cc j", p=PDIM))
        for sp in range(NSPAN):
            for h in range(H):
                if sp == 0 and h in (1, 3, 5):
                    hp = (h + 1) // 2
                    emit_kT([hp])
                    emit_qT(queryT0, qTs=qTs, hps=[hp])
                    if h == 5:
                        ctxKV.close()
                if h == 1 and sp + 1 < NSPAN:
                    Ls_next = emit_L(sp + 1)
                if h == 2 and sp + 1 < NSPAN:
                    queryT_next = emit_queryT(sp + 1)
                if h == 4 and sp + 1 < NSPAN:
                    qTs_next = emit_qT(queryT_next)
                flush_yT()
                if h == 1 and sp >= 1:
                    emit_outproj(sp - 1)
                hq = qTs[(h % 2) * Dh:(h % 2 + 1) * Dh, h // 2, :]
                hk = kT[(h % 2) * Dh:(h % 2 + 1) * Dh, h // 2, :]
                e_tiles = []
                zt = poolZ.tile([PDIM, JJ], f32, tag="z")
                rz = poolZ.tile([PDIM, JJ], f32, tag="rz")
                for jj in range(JJ):
                    jt = sp * JJ + jj
                    # score path
                    pss = ps_s.tile([PDIM, T2], f32, tag="ps")
                    nc.tensor.matmul(
                        pss[:],
                        hq[:, jj * PDIM:(jj + 1) * PDIM],
                        hk,
                        start=True,
                        stop=True,
                    )
                    s_sb = poolS.tile([PDIM, T2], f32)
                    nc.scalar.copy(s_sb[:], pss[:])
                    nc.sync.dma_start(
                        score[h, jt * PDIM:(jt + 1) * PDIM, :], s_sb[:]
                    )
                    emit_pending_prob(1)
                    # exp path: recompute s, add L, exp (+Z via accum_out)
                    pse = ps_e.tile([PDIM, T2], f32, tag="pe")
                    nc.tensor.matmul(
                        pse[:],
                        hq[:, jj * PDIM:(jj + 1) * PDIM],
                        hk,
                        start=True,
                        stop=False,
                    )
                    nc.tensor.matmul(
                        pse[:],
                        identr[:],
                        Ls_cur[:, jj, :],
                        start=False,
                        stop=True,
                    )
                    e_sb = poolE.tile([PDIM, T2], f32)
                    nc.scalar.activation(
                        e_sb[:], pse[:], AF.Exp,
                        accum_out=zt[:, jj:jj + 1],
                    )
                    # per-tile reciprocal so the normalize of tile jj only
                    # waits on its own exp, not all four
                    nc.vector.reciprocal(rz[:, jj:jj + 1], zt[:, jj:jj + 1])
                    e_tiles.append(e_sb)
                if sp == 0 and h == 0:
                    emit_v()
                ptT = poolPT.tile([PDIM, NTC, 512], f32r)
                for jj in range(JJ):
                    jt = sp * JJ + jj
                    p_sb = poolP.tile([PDIM, T2], f32r)
                    nc.vector.tensor_scalar_mul(
                        p_sb[:], e_tiles[jj][:], rz[:, jj:jj + 1]
                    )
                    pending_probs.append((p_sb, h, jt))
                    pst = ps_t.tile([PDIM, T2], f32, tag="pt")
                    for tc_ in range(NTC):
                        nc.tensor.transpose(
                            pst[:, tc_ * PDIM:(tc_ + 1) * PDIM].bitcast(f32r),
                            p_sb[:, tc_ * PDIM:(tc_ + 1) * PDIM],
                            identr[:],
                        )
                    nc.vector.tensor_copy(
                        ptT[:, :, jj * PDIM:(jj + 1) * PDIM],
                        pst[:].rearrange("p (tc t) -> p tc t", t=PDIM),
                    )
                # y^T_h[span] = v_h^T @ P^T
                psy = ps_y.tile([Dh, 512], f32, tag="py")
                for tc_ in range(NTC):
                    nc.tensor.matmul(
                        psy[:],
                        v_sb[:, tc_, h * Dh:(h + 1) * Dh],
                        ptT[:, tc_, :],
                        start=(tc_ == 0),
                        stop=(tc_ == NTC - 1),
                    )
                pending_yT[0] = (psy, h, sp)

            if sp + 1 < NSPAN:
                qTs = qTs_next
                Ls_cur = Ls_next
        emit_pending_prob(len(pending_probs))
        flush_yT()
        emit_outproj(NSPAN - 1)

    nc.compile()
    bacc.get_activation_tables = _orig_tabs
    return nc


_NC_CACHE = {}


def kernel(query, query_mask, memory, memory_mask, attn_prior, Wq, Wkv, Wo):
    from concourse.bass_utils import run_bass_kernel_spmd

    query = np.asarray(query, dtype=np.float32)
    memory = np.asarray(memory, dtype=np.float32)
    attn_prior = np.asarray(attn_prior, dtype=np.float32)
    scale = np.float32(Dh ** -0.5)
    wq_s = np.ascontiguousarray(np.asarray(Wq, dtype=np.float32) * scale)
    wkv_h = np.ascontiguousarray(np.asarray(Wkv, dtype=np.float32))
    wo_h = np.ascontiguousarray(np.asarray(Wo, dtype=np.float32))

    if "nc" not in _NC_CACHE:
        _NC_CACHE["nc"] = _build_nc()
    nc = _NC_CACHE["nc"]

    in_maps = [
        {
            "query": np.ascontiguousarray(query[b]),
            "memory": np.ascontiguousarray(memory[b]),
            "prior": np.ascontiguousarray(attn_prior[b]),
            "wq": wq_s,
            "wkv": wkv_h,
            "wo": wo_h,
        }
        for b in range(B)
    ]
    res = run_bass_kernel_spmd(nc, in_maps, core_ids=list(range(B)))
    _NC_CACHE["last_result"] = res
    out = np.stack([r["out"] for r in res.results])
    prob = np.stack([r["prob"] for r in res.results])
    score = np.stack([r["score"] for r in res.results])
    return out, prob, score
